# revision 64
# baseline (speedup 1.0000x reference)
"""Trainium2 Bass kernel for batched tanh-query attention.

Per-batch computation (B=8, one batch per NeuronCore, pure data parallel):
    q = tanh(out_state)            [Q, H]    Q=K=2048, H=128
    S = q @ history.T              [Q, K]
    P = softmax(S, axis=K)
    attn = P @ history             [Q, H]

Flash-style in the transposed orientation S_T[k, q] (no transpose of P
needed for the second matmul). Queries processed in 4 quarters of 512
columns; within a quarter the 16 k-tiles are processed in 6 GROUPS
(2/2/3/3/3/3 tiles) so each exp instruction covers [128, 512*g] —
wider activations amortize the ~352-cycle fixed ACTIVATE overhead
(the Activation engine is the bottleneck: exp at 1 elem/lane/cycle).
    PE : MM1 group g+2 | MM2 group g-1 (accumulating matmuls)
    ACT: exp over st[128, 512*g] f32 PSUM -> ex bf16 SBUF
    DVE: running bf16 adds for the softmax denominator, epilogue
    Pool: f32->bf16 conversion of late history chunks
PSUM budget (8 banks): st x2 (3 banks each) + acc x1 + shared bank
holding both the PE-transpose staging slots and the bf16 denominator
columns (computed by transpose-mode matmuls with a ones vector).
"""

import os
import sys

os.environ.setdefault("NEURON_RT_RESET_CORES", "1")
for _p in ("/opt/trn_rl_repo", "/opt/trn_rl_repo/concourse"):
    if _p not in sys.path:
        sys.path.insert(0, _p)

import numpy as np

N_CORES = 8
SEQ = 2048
H = 128
P = 128
T = SEQ // P          # 16 seq tiles
NQ = 4                # query quarters
QW = SEQ // NQ        # 512
QTPQ = QW // P        # 4 q-tiles per quarter
NG = 6                # k-tile groups per quarter

# group layouts: (start_tile, n_tiles) per group
# early quarters ramp with small groups (fewer ht tiles needed at start);
# the last quarter ends with small groups (short post-exp tail)
GROUPS_EARLY = [(0, 2), (2, 2), (4, 3), (7, 3), (10, 3), (13, 3)]
GROUPS_LAST = [(0, 3), (3, 3), (6, 3), (9, 3), (12, 2), (14, 2)]

_CACHE = {}


def _build():
    from concourse import bacc, bass, masks, mybir, tile

    f32 = mybir.dt.float32
    bf16 = mybir.dt.bfloat16
    AF = mybir.ActivationFunctionType

    debug_dump = os.environ.get("KERNEL_DEBUG_DUMP", "0") == "1"
    EX_BUFS = 24 if debug_dump else 6

    nc = bacc.Bacc("TRN2", target_bir_lowering=False, debug=False,
                   num_devices=N_CORES)
    os_d = nc.dram_tensor("out_state", (SEQ, H), f32, kind="ExternalInput")
    h_d = nc.dram_tensor("history", (SEQ, H), f32, kind="ExternalInput")
    a_d = nc.dram_tensor("attn", (SEQ, H), f32, kind="ExternalOutput")
    if debug_dump:
        dbg = {
            "qnat": nc.dram_tensor("dbg_qnat", (SEQ, H), mybir.dt.bfloat16,
                                   kind="ExternalOutput"),
            "qT": nc.dram_tensor("dbg_qT", (SEQ, H), mybir.dt.bfloat16,
                                 kind="ExternalOutput"),
            "ht": nc.dram_tensor("dbg_ht", (SEQ, H), mybir.dt.bfloat16,
                                 kind="ExternalOutput"),
            "hn": nc.dram_tensor("dbg_hn", (SEQ, H), mybir.dt.bfloat16,
                                 kind="ExternalOutput"),
            "rc": nc.dram_tensor("dbg_rc", (P, 16), f32,
                                 kind="ExternalOutput"),
            "attn2": nc.dram_tensor("dbg_attn2", (SEQ, H), f32,
                                    kind="ExternalOutput"),
            "aT": nc.dram_tensor("dbg_aT", (P, 2048), mybir.dt.bfloat16,
                                 kind="ExternalOutput"),
            "tf": nc.dram_tensor("dbg_tf", (P, 2048), mybir.dt.bfloat16,
                                 kind="ExternalOutput"),
            "ex": nc.dram_tensor("dbg_ex", (P, 24 * 1536), mybir.dt.bfloat16,
                                 kind="ExternalOutput"),
        }

    with tile.TileContext(nc) as tc:
        with (
            tc.tile_pool(name="const", bufs=1) as constp,
            tc.tile_pool(name="big", bufs=1) as bigp,
            tc.tile_pool(name="ex", bufs=EX_BUFS) as expool,
            tc.tile_pool(name="tree", bufs=3) as treep,
            tc.tile_pool(name="work", bufs=4) as workp,
            tc.tile_pool(name="rcp", bufs=4) as rcp,
            tc.tile_pool(name="psst", bufs=2, space=bass.MemorySpace.PSUM) as psst,
            tc.tile_pool(name="pacc", bufs=1, space=bass.MemorySpace.PSUM) as pacc,
            tc.tile_pool(name="pstp", bufs=1, space=bass.MemorySpace.PSUM) as pstp,
        ):
            # ---- input DMAs first so transfers overlap const setup ----
            os_f = bigp.tile([P, T, H], f32, tag="osf")
            hn_f = bigp.tile([P, T, H], f32, tag="hnf")
            os_v = os_d[:].rearrange("(t p) h -> p t h", p=P)
            hn_v = h_d[:].rearrange("(t p) h -> p t h", p=P)
            a_v = a_d[:].rearrange("(t p) h -> p t h", p=P)
            # Two HWDGE queues in parallel, priority order per queue.  Early
            # chunks are latency-bound (~1.5-2us each regardless of size),
            # so use few, large chunks: os on sync, hn on scalar.  The
            # scalar queue's automatic ACT table load then lands after its
            # dma issues, right before the first tanh needs it.
            # (gpsimd dma_start is SWDGE — avoid.)
            nc.sync.dma_start(os_f[:, 0:4, :], os_v[:, 0:4, :])
            nc.scalar.dma_start(hn_f[:, 0:4, :], hn_v[:, 0:4, :])
            nc.sync.dma_start(hn_f[:, 4:6, :], hn_v[:, 4:6, :])
            nc.sync.dma_start(hn_f[:, 6:8, :], hn_v[:, 6:8, :])
            nc.scalar.dma_start(hn_f[:, 8:13, :], hn_v[:, 8:13, :])
            nc.sync.dma_start(os_f[:, 4:10, :], os_v[:, 4:10, :])
            nc.sync.dma_start(hn_f[:, 13:16, :], hn_v[:, 13:16, :])
            nc.sync.dma_start(os_f[:, 10:16, :], os_v[:, 10:16, :])

            id_bf = constp.tile([P, P], bf16, tag="idb")
            masks.make_identity(nc, id_bf[:])
            ones_bf = constp.tile([P, 1], bf16, tag="ones")
            nc.vector.memset(ones_bf[:], 1.0)

            # persistent bf16 operands
            hn = bigp.tile([P, T, P], bf16, tag="hn")    # [k_in, t, h] natural
            ht = bigp.tile([P, T, P], bf16, tag="ht")    # [h, t, k_in]
            qT = bigp.tile([P, T, P], bf16, tag="qT")    # [h, t, q_in]
            q_nat = bigp.tile([P, T, H], bf16, tag="qnat")
            ot_all = bigp.tile([P, T, H], f32, tag="ot")  # output staging

            # ---- prologue compute ----
            # earliest-needed history tiles converted on DVE (fast); the
            # latest tiles go to the otherwise-idle Pool engine
            nc.vector.tensor_copy(hn[:, 0:4, :], hn_f[:, 0:4, :])
            nc.scalar.activation(q_nat[:, 0:4, :], os_f[:, 0:4, :], AF.Tanh)
            nc.gpsimd.tensor_copy(hn[:, 13:16, :], hn_f[:, 13:16, :])

            # PE warm-up: the tensor engine ramps its clock only after ~3us
            # of continuous work; burn the DMA-wait window with dummy
            # transposes so the first real matmuls run at full speed
            wrm = pstp.tile([P, 8, P], bf16, tag="tsp", name="warm")
            for i in range(24):
                nc.tensor.transpose(wrm[:, i % 8, :], id_bf[:], id_bf[:])
            # a few late-hinted dummies keep the PE clock up through the
            # DMA-wait window until the first real matmuls
            with tc.tile_wait_until(0.003):
                for i in range(6):
                    nc.tensor.transpose(wrm[:, i, :], id_bf[:], id_bf[:])
            with tc.tile_wait_until(0.0042):
                for i in range(6):
                    nc.tensor.transpose(wrm[:, i, :], id_bf[:], id_bf[:])
            wrm_keep = constp.tile([P, 1], bf16, tag="wk")
            nc.vector.tensor_copy(wrm_keep[:], wrm[:, 0, 0:1])

            # one batch of PE transposes through the shared tsp PSUM bank,
            # then one DVE copy out per destination range
            def tp_batch(jobs):
                # jobs: list of (dst_tile, dst_t0, src_tile, src_t0, n)
                tsp = pstp.tile([P, 8, P], bf16, tag="tsp", name="tsp")
                s = 0
                for dst, dt0, src, st0, n in jobs:
                    for i in range(n):
                        nc.tensor.transpose(tsp[:, s + i, :],
                                            src[:, st0 + i, :], id_bf[:])
                    s += n
                s = 0
                for dst, dt0, src, st0, n in jobs:
                    nc.vector.tensor_copy(dst[:, dt0:dt0 + n, :],
                                          tsp[:, s:s + n, :])
                    s += n

            # first MM1 inputs: ht[0:4] (after its DVE cast) and qT[0:4]
            tp_batch([(ht, 0, hn, 0, 4), (qT, 0, q_nat, 0, 4)])

            def groups_of(q):
                return GROUPS_LAST if q == NQ - 1 else GROUPS_EARLY

            # ---- pipeline emitters ----
            sts = {}          # (q, g) -> st tile
            exs = {}          # (q, g) -> ex tile
            accs = [None] * NQ

            def emit_mm1(q, g):
                k0, n = groups_of(q)[g]
                st = psst.tile([P, 3 * QW], f32, tag="st", name=f"st{q}{g}")
                rhs = qT[:, QTPQ * q:QTPQ * (q + 1), :]
                for j in range(n):
                    nc.tensor.matmul(st[:, QW * j:QW * (j + 1)],
                                     ht[:, k0 + j, :], rhs,
                                     start=True, stop=True)
                sts[(q, g)] = st

            def emit_exp(q, g, split=False):
                k0, n = groups_of(q)[g]
                st = sts.pop((q, g))
                ex = expool.tile([P, 3 * QW], bf16, tag="ex", name=f"ex{q}{g}")
                if split:
                    # split the very last exp so the final MM2 (and with it
                    # the whole output tail) starts half an exp earlier
                    nc.scalar.activation(ex[:, 0:QW], st[:, 0:QW], AF.Exp)
                    nc.scalar.activation(ex[:, QW:QW * n], st[:, QW:QW * n],
                                         AF.Exp)
                else:
                    nc.scalar.activation(ex[:, 0:QW * n], st[:, 0:QW * n],
                                         AF.Exp)
                exs[(q, g)] = ex
                dbg_ex.append(ex)

            def emit_mm2(q, g):
                k0, n = groups_of(q)[g]
                if accs[q] is None:
                    accs[q] = pacc.tile([P, QW], f32, tag="acc",
                                        name=f"acc{q}")
                ex = exs[(q, g)]
                for j in range(n):
                    kb = k0 + j
                    nc.tensor.matmul(accs[q][:], hn[:, kb, :],
                                     ex[:, QW * j:QW * (j + 1)],
                                     start=(kb == 0), stop=(kb == T - 1))

            # ---- denominator tree (bf16 adds on DVE) ----
            # state per quarter: pair tiles and the 512-wide running sum
            tstate = {}

            def tadd(name, w, a, b, tag=None, bufs=None):
                # running-sum tiles need bufs=3: S_{i+1} reads S_i, so with
                # 2 bufs the new tile would land on the buffer its own add
                # is reading
                t = treep.tile([P, w], bf16, tag=tag or f"tr{w}", name=name,
                               bufs=bufs)
                nc.vector.tensor_add(t[:], a, b)
                return t

            def sadd(name, a, b):
                return tadd(name, QW, a, b, tag="trS", bufs=3)

            def tree_step(q, g):
                # called after emit_exp(q, g); updates running denominator
                st8 = tstate.setdefault(q, {})
                if q < NQ - 1:
                    # groups 2/2/3/3/3/3
                    if g == 1:
                        st8["t4"] = tadd(f"t4_{q}", 2 * QW,
                                         exs[(q, 0)][:, 0:2 * QW],
                                         exs[(q, 1)][:, 0:2 * QW])
                    elif g == 2:
                        t4 = st8.pop("t4")
                        st8["S"] = sadd(f"s1_{q}", t4[:, 0:QW], t4[:, QW:])
                    elif g == 3:
                        st8["t1"] = tadd(f"t1_{q}", 3 * QW,
                                         exs[(q, 2)][:], exs[(q, 3)][:])
                    elif g == 4:
                        t1 = st8.pop("t1")
                        f1 = tadd(f"f1_{q}", QW, t1[:, 0:QW], t1[:, QW:2 * QW],
                                  tag="trf")
                        s2 = sadd(f"s2_{q}", st8["S"][:], f1[:])
                        st8["S"] = sadd(f"s3_{q}", s2[:], t1[:, 2 * QW:])
                    elif g == 5:
                        st8["t2"] = tadd(f"t2_{q}", 3 * QW,
                                         exs[(q, 4)][:], exs[(q, 5)][:])
                else:
                    # groups 3/3/3/3/2/2.  The denominator is finished by
                    # PE matmul accumulation (the PE has tail slack, DVE
                    # does not): only G0+G1 are folded to a 512-wide S1;
                    # t2 (G2+G3), the G4 fold v, and the G5 fold w feed the
                    # dc accumulation directly.
                    if g == 1:
                        st8["t1"] = tadd(f"t1_{q}", 3 * QW,
                                         exs[(q, 0)][:], exs[(q, 1)][:])
                    elif g == 2:
                        t1 = st8.pop("t1")
                        f1 = tadd(f"f1_{q}", QW, t1[:, 0:QW], t1[:, QW:2 * QW],
                                  tag="trf")
                        st8["S"] = sadd(f"s1_{q}", f1[:], t1[:, 2 * QW:])
                    elif g == 3:
                        st8["t2"] = tadd(f"t2_{q}", 3 * QW,
                                         exs[(q, 2)][:], exs[(q, 3)][:])
                    elif g == 4:
                        t2 = st8.pop("t2")
                        f2 = tadd(f"f2_{q}", QW, t2[:, 0:QW], t2[:, QW:2 * QW],
                                  tag="trf")
                        s2 = sadd(f"s2_{q}", st8["S"][:], f2[:])
                        st8["S"] = sadd(f"s3_{q}", s2[:], t2[:, 2 * QW:])
                    elif g == 5:
                        st8["v"] = tadd(f"v_{q}", QW, exs[(q, 4)][:, 0:QW],
                                        exs[(q, 4)][:, QW:2 * QW], tag="trf")

            def tree_finish(q):
                # mid quarters: fold the last pair tile into the running sum
                st8 = tstate[q]
                t2 = st8.pop("t2")
                f2 = tadd(f"f2_{q}", QW, t2[:, 0:QW], t2[:, QW:2 * QW],
                          tag="trf")
                s4 = sadd(f"s4_{q}", st8["S"][:], f2[:])
                st8["tfin"] = tadd(f"tf_{q}", QW, s4[:], t2[:, 2 * QW:],
                                   tag="tf", bufs=4)
                dbg_tf.append(st8["tfin"])

            def tree_finish_last(q):
                st8 = tstate[q]
                w = tadd(f"w_{q}", QW, exs[(q, 5)][:, 0:QW],
                         exs[(q, 5)][:, QW:2 * QW], tag="trf")
                st8["tfin"] = tadd(f"tf_{q}", QW, st8["S"][:], w[:], tag="tf",
                                   bufs=4)
                dbg_tf.append(st8["tfin"])

            # ---- epilogue helper: one quarter's outputs ----
            def make_epilogue(q):
                st8 = {}

                def dve_copy():
                    aT = workp.tile([P, QW], bf16, tag="at", name=f"aT{q}")
                    if q == NQ - 1:
                        # split between DVE and the now-idle ACT engine so
                        # the tail's transposes start half a copy earlier
                        nc.vector.tensor_copy(aT[:, 0:QW // 2],
                                              accs[q][:, 0:QW // 2])
                        nc.scalar.activation(aT[:, QW // 2:],
                                             accs[q][:, QW // 2:], AF.Copy)
                    else:
                        nc.vector.tensor_copy(aT[:], accs[q][:])
                    st8["aT"] = aT
                    dbg_aT.append(aT)

                def pe_dcols():
                    # d-columns via 1-col matmuls with a ones vector (the
                    # partition-dim sum of tfin).  The dc tile reuses the acc
                    # bank: after the aT copy the accumulator is dead, and
                    # pool WAR ordering (copy -> dc -> recip -> next acc)
                    # keeps PE writes and DVE reads of the bank apart.
                    dc = pacc.tile([P, QW], f32, tag="acc", name=f"dc{q}")
                    tfin = tstate[q]["tfin"]
                    for t in range(QTPQ):
                        nc.tensor.matmul(dc[:, t:t + 1],
                                         tfin[:, P * t:P * (t + 1)],
                                         ones_bf[:], start=True, stop=True)
                    st8["dc"] = dc

                def dve_recip():
                    rc = rcp.tile([P, QTPQ], f32, tag="rc", name=f"rc{q}")
                    nc.vector.reciprocal(rc[:], st8["dc"][:, 0:QTPQ])
                    st8["rc"] = rc
                    rc_tiles.append(rc)

                def pe_transposes():
                    ep = pstp.tile([P, 8, P], bf16, tag="tsp", name=f"ep{q}")
                    st8["ep"] = ep
                    for t in range(QTPQ):
                        nc.tensor.transpose(ep[:, t, :],
                                            st8["aT"][:, P * t:P * (t + 1)],
                                            id_bf[:])

                def dve_muls(ts, on_act=False):
                    for t in ts:
                        if on_act:
                            # ACT is idle after the last exp; Copy-with-scale
                            # halves the tail's serial mul chain
                            nc.scalar.activation(
                                ot_all[:, QTPQ * q + t, :],
                                st8["ep"][:, t, :], AF.Copy,
                                scale=st8["rc"][:, t:t + 1])
                        else:
                            nc.vector.tensor_scalar_mul(
                                ot_all[:, QTPQ * q + t, :],
                                st8["ep"][:, t, :], st8["rc"][:, t:t + 1])

                def dma_out(t0, t1, queue=None):
                    (queue or nc.sync).dma_start(
                        a_v[:, QTPQ * q + t0:QTPQ * q + t1, :],
                        ot_all[:, QTPQ * q + t0:QTPQ * q + t1, :])

                st8["fns"] = (dve_copy, pe_dcols, dve_recip, pe_transposes,
                              dve_muls, dma_out)
                return st8

            # ---- main pipeline ----
            epi = [None] * NQ
            rc_tiles = []
            dbg_ex = []
            dbg_aT = []
            dbg_tf = []

            dc3 = None

            for q in range(NQ):
                last = q == NQ - 1
                for g in range(NG):
                    if q == 0 and g == 0:
                        emit_mm1(0, 0)
                        emit_mm1(0, 1)
                    emit_exp(q, g, split=(last and g == NG - 1))
                    tree_step(q, g)
                    if last and g == NG - 1:
                        # dc_a: accumulate S3 (G0..G3) into a dc tile in the
                        # now-idle st pool while exp(G5) runs; the G4 fold v
                        # and the G5 fold w complete the accumulation later.
                        # Only the FIRST matmul may set start: a start clears
                        # has_written for the WHOLE bank, which would wipe
                        # the other columns' accumulate bits.
                        dc3 = psst.tile([P, 3 * QW], f32, tag="st",
                                        name="dc3")
                        s3 = tstate[q]["S"]
                        for t in range(QTPQ):
                            nc.tensor.matmul(dc3[:, t:t + 1],
                                             s3[:, P * t:P * (t + 1)],
                                             ones_bf[:], start=(t == 0),
                                             stop=False,
                                             skip_group_check=True)

                    if q == 0:
                        # history cast + transpose chains for later groups,
                        # and tanh batches for later quarters; qT(q1) must be
                        # fully emitted before the hoisted MM1(q1, 0) at the
                        # end of slot 4 (PE queue is in-order)
                        if g == 0:
                            with tc.tile_wait_until(0.0055):
                                nc.vector.tensor_copy(hn[:, 4:6, :],
                                                      hn_f[:, 4:6, :])
                            tp_batch([(ht, 4, hn, 4, 2)])
                            with tc.tile_wait_until(0.006):
                                nc.vector.tensor_copy(hn[:, 6:8, :],
                                                      hn_f[:, 6:8, :])
                            tp_batch([(ht, 6, hn, 6, 2)])
                        elif g == 1:
                            with tc.tile_wait_until(0.0055):
                                nc.vector.tensor_copy(hn[:, 8:10, :],
                                                      hn_f[:, 8:10, :])
                            tp_batch([(ht, 8, hn, 8, 2)])
                        elif g == 2:
                            with tc.tile_wait_until(0.008):
                                nc.scalar.activation(q_nat[:, 4:10, :],
                                                     os_f[:, 4:10, :],
                                                     AF.Tanh)
                            with tc.tile_wait_until(0.0095):
                                nc.vector.tensor_copy(hn[:, 10:13, :],
                                                      hn_f[:, 10:13, :])
                            tp_batch([(ht, 10, hn, 10, 3)])
                        elif g == 3:
                            tp_batch([(ht, 13, hn, 13, 3),
                                      (qT, QTPQ, q_nat, QTPQ, 2)])
                        elif g == 4:
                            with tc.tile_wait_until(0.014):
                                nc.scalar.activation(q_nat[:, 10:16, :],
                                                     os_f[:, 10:16, :],
                                                     AF.Tanh)
                            tp_batch([(qT, QTPQ + 2, q_nat, QTPQ + 2, 2)])
                    elif epi[q - 1] is not None:
                        # epilogue of the previous quarter, spread over slots
                        st8 = epi[q - 1]
                        dve_copy, pe_dcols, dve_recip, pe_trans, dve_muls, \
                            dma_out = st8["fns"]
                        if g == 0:
                            dve_copy()
                        elif g == 1:
                            pe_dcols()
                            dve_recip()
                            # deferred first MM2 of this quarter: its acc
                            # allocation now orders after dc/recip above
                            emit_mm2(q, 0)
                        elif g == 2:
                            pe_trans()
                            dve_muls([0, 1])
                        elif g == 3:
                            dve_muls([2, 3])
                            dma_out(0, 4)
                            epi[q - 1] = None
                            if q < NQ - 1:
                                # before the hoisted MM1(q+1, 0) below
                                tp_batch([(qT, QTPQ * (q + 1), q_nat,
                                           QTPQ * (q + 1), 4)])

                    # leading MM1s, then the lagging MM2 — the scheduler
                    # prefers earlier-priority PE work when both are ready,
                    # and the MM1s feed the exp stream while MM2s have slack
                    if g + 2 < NG:
                        emit_mm1(q, g + 2)
                    elif g + 2 == NG and q < NQ - 1:
                        emit_mm1(q + 1, 0)
                    if g >= 1 and not (g == 1 and q >= 1):
                        # MM2(q, 0) of quarters >= 1 was emitted with the
                        # epilogue extras (acc-bank allocation ordering)
                        emit_mm2(q, g - 1)

                # close the quarter: the hoisted MM1(q+1, 1) goes BEFORE
                # MM2(q, 5) — both wait on exp(q, 5), but the MM1 feeds the
                # next quarter's exp stream while the MM2 has slack
                if not last:
                    emit_mm1(q + 1, 1)
                if last:
                    # dc columns for the G4 fold v: ready before exp(G5)
                    # ends, so they run ahead of MM2(G5) on the PE queue
                    v = tstate[q]["v"]
                    for t in range(QTPQ):
                        nc.tensor.matmul(dc3[:, t:t + 1],
                                         v[:, P * t:P * (t + 1)],
                                         ones_bf[:], start=False, stop=False,
                                         skip_group_check=True)
                emit_mm2(q, NG - 1)
                if not last:
                    tree_finish(q)
                    epi[q] = make_epilogue(q)

            # ---- final quarter tail, emitted tight ----
            q = NQ - 1
            st8 = make_epilogue(q)
            dve_copy, pe_dcols, dve_recip, pe_trans, dve_muls, dma_out = \
                st8["fns"]
            # fold the last group's two chunks, finish the dc accumulation,
            # and take the reciprocal — overlapping the aT copy + transposes
            w = tadd(f"w_{q}", QW, exs[(q, NG - 1)][:, 0:QW],
                     exs[(q, NG - 1)][:, QW:2 * QW], tag="trf")
            dve_copy()
            for t in range(QTPQ):
                nc.tensor.matmul(dc3[:, t:t + 1], w[:, P * t:P * (t + 1)],
                                 ones_bf[:], start=False, stop=(t == QTPQ - 1),
                                 skip_group_check=True)
            rc3 = rcp.tile([P, QTPQ], f32, tag="rc", name="rc3")
            nc.vector.reciprocal(rc3[:], dc3[:, 0:QTPQ])
            st8["rc"] = rc3
            rc_tiles.append(rc3)
            pe_trans()
            dve_muls([0])
            dma_out(0, 1, queue=nc.sync)
            dve_muls([2], on_act=True)
            dma_out(2, 3, queue=nc.scalar)
            dve_muls([1])
            dma_out(1, 2, queue=nc.sync)
            dve_muls([3])
            dma_out(3, 4, queue=nc.scalar)

            if debug_dump:
                for key, src in (("qnat", q_nat), ("qT", qT), ("ht", ht),
                                 ("hn", hn)):
                    nc.sync.dma_start(
                        dbg[key][:].rearrange("(t p) h -> p t h", p=P), src[:])
                nc.sync.dma_start(dbg["attn2"][:].rearrange(
                    "(t p) h -> p t h", p=P), ot_all[:])
                for i, rc in enumerate(rc_tiles):
                    nc.sync.dma_start(dbg["rc"][:, 4 * i:4 * (i + 1)], rc[:])
                for i, at in enumerate(dbg_aT):
                    nc.sync.dma_start(dbg["aT"][:, 512 * i:512 * (i + 1)],
                                      at[:])
                for i, tf in enumerate(dbg_tf):
                    nc.sync.dma_start(dbg["tf"][:, 512 * i:512 * (i + 1)],
                                      tf[:])
                for i, ex in enumerate(dbg_ex):
                    nc.sync.dma_start(dbg["ex"][:, 1536 * i:1536 * (i + 1)],
                                      ex[:])

    nc.compile()
    return nc


def _get_nc():
    if "nc" not in _CACHE:
        _CACHE["nc"] = _build()
    return _CACHE["nc"]


def _run(out_state, history, trace=False):
    from concourse.bass_utils import run_bass_kernel_spmd

    nc = _get_nc()
    out_state = np.ascontiguousarray(out_state, dtype=np.float32)
    history = np.ascontiguousarray(history, dtype=np.float32)
    in_maps = [
        {"out_state": out_state[b], "history": history[b]}
        for b in range(N_CORES)
    ]
    if "warmed" not in _CACHE:
        # The very first execution after NEFF load can start with polluted
        # engine semaphores (the loader's DMA activity bumps them), letting
        # consumers race ahead of producers.  The program's teardown clears
        # all semaphores, so execute once and discard; every execution
        # after that is clean.
        run_bass_kernel_spmd(nc, in_maps, core_ids=list(range(N_CORES)))
        _CACHE["warmed"] = True
    res = run_bass_kernel_spmd(nc, in_maps, core_ids=list(range(N_CORES)),
                               trace=trace)
    attn = np.stack([res.results[b]["attn"] for b in range(N_CORES)], axis=0)
    return attn.astype(np.float32), res


def kernel(out_state, history):
    try:
        attn, _ = _run(out_state, history)
    except Exception:
        # one retry, e.g. if a previous process left a core wedged
        attn, _ = _run(out_state, history)
    return attn


# revision 65
# speedup vs baseline: 1.0200x; 1.0200x over previous
"""Trainium2 Bass kernel for batched tanh-query attention.

Per-batch computation (B=8, one batch per NeuronCore, pure data parallel):
    q = tanh(out_state)            [Q, H]    Q=K=2048, H=128
    S = q @ history.T              [Q, K]
    P = softmax(S, axis=K)
    attn = P @ history             [Q, H]

Flash-style in the transposed orientation S_T[k, q] (no transpose of P
needed for the second matmul). Queries processed in 4 quarters of 512
columns; within a quarter the 16 k-tiles are processed in 6 GROUPS
(2/2/3/3/3/3 tiles) so each exp instruction covers [128, 512*g] —
wider activations amortize the ~352-cycle fixed ACTIVATE overhead
(the Activation engine is the bottleneck: exp at 1 elem/lane/cycle).
    PE : MM1 group g+2 | MM2 group g-1 (accumulating matmuls)
    ACT: exp over st[128, 512*g] f32 PSUM -> ex bf16 SBUF
    DVE: running bf16 adds for the softmax denominator, epilogue
    Pool: f32->bf16 conversion of late history chunks
PSUM budget (8 banks): st x2 (3 banks each) + acc x1 + shared bank
holding both the PE-transpose staging slots and the bf16 denominator
columns (computed by transpose-mode matmuls with a ones vector).
"""

import os
import sys

os.environ.setdefault("NEURON_RT_RESET_CORES", "1")
for _p in ("/opt/trn_rl_repo", "/opt/trn_rl_repo/concourse"):
    if _p not in sys.path:
        sys.path.insert(0, _p)

import numpy as np

N_CORES = 8
SEQ = 2048
H = 128
P = 128
T = SEQ // P          # 16 seq tiles
NQ = 4                # query quarters
QW = SEQ // NQ        # 512
QTPQ = QW // P        # 4 q-tiles per quarter
NG = 6                # k-tile groups per quarter

# group layouts: (start_tile, n_tiles) per group
# early quarters ramp with small groups (fewer ht tiles needed at start);
# the last quarter ends with small groups (short post-exp tail)
GROUPS_EARLY = [(0, 2), (2, 2), (4, 3), (7, 3), (10, 3), (13, 3)]
GROUPS_LAST = [(0, 3), (3, 3), (6, 3), (9, 3), (12, 2), (14, 2)]

_CACHE = {}


def _build():
    from concourse import bacc, bass, masks, mybir, tile

    f32 = mybir.dt.float32
    bf16 = mybir.dt.bfloat16
    AF = mybir.ActivationFunctionType

    debug_dump = os.environ.get("KERNEL_DEBUG_DUMP", "0") == "1"
    EX_BUFS = 24 if debug_dump else 6

    nc = bacc.Bacc("TRN2", target_bir_lowering=False, debug=False,
                   num_devices=N_CORES)
    os_d = nc.dram_tensor("out_state", (SEQ, H), f32, kind="ExternalInput")
    h_d = nc.dram_tensor("history", (SEQ, H), f32, kind="ExternalInput")
    a_d = nc.dram_tensor("attn", (SEQ, H), f32, kind="ExternalOutput")
    if debug_dump:
        dbg = {
            "qnat": nc.dram_tensor("dbg_qnat", (SEQ, H), mybir.dt.bfloat16,
                                   kind="ExternalOutput"),
            "qT": nc.dram_tensor("dbg_qT", (SEQ, H), mybir.dt.bfloat16,
                                 kind="ExternalOutput"),
            "ht": nc.dram_tensor("dbg_ht", (SEQ, H), mybir.dt.bfloat16,
                                 kind="ExternalOutput"),
            "hn": nc.dram_tensor("dbg_hn", (SEQ, H), mybir.dt.bfloat16,
                                 kind="ExternalOutput"),
            "rc": nc.dram_tensor("dbg_rc", (P, 16), f32,
                                 kind="ExternalOutput"),
            "attn2": nc.dram_tensor("dbg_attn2", (SEQ, H), f32,
                                    kind="ExternalOutput"),
            "aT": nc.dram_tensor("dbg_aT", (P, 2048), mybir.dt.bfloat16,
                                 kind="ExternalOutput"),
            "tf": nc.dram_tensor("dbg_tf", (P, 2048), mybir.dt.bfloat16,
                                 kind="ExternalOutput"),
            "ex": nc.dram_tensor("dbg_ex", (P, 24 * 1536), mybir.dt.bfloat16,
                                 kind="ExternalOutput"),
        }

    with tile.TileContext(nc) as tc:
        with (
            tc.tile_pool(name="const", bufs=1) as constp,
            tc.tile_pool(name="big", bufs=1) as bigp,
            tc.tile_pool(name="ex", bufs=EX_BUFS) as expool,
            tc.tile_pool(name="tree", bufs=3) as treep,
            tc.tile_pool(name="work", bufs=4) as workp,
            tc.tile_pool(name="rcp", bufs=4) as rcp,
            tc.tile_pool(name="psst", bufs=2, space=bass.MemorySpace.PSUM) as psst,
            tc.tile_pool(name="pacc", bufs=1, space=bass.MemorySpace.PSUM) as pacc,
            tc.tile_pool(name="pstp", bufs=1, space=bass.MemorySpace.PSUM) as pstp,
        ):
            # ---- input DMAs first so transfers overlap const setup ----
            os_f = bigp.tile([P, T, H], f32, tag="osf")
            hn_f = bigp.tile([P, T, H], f32, tag="hnf")
            os_v = os_d[:].rearrange("(t p) h -> p t h", p=P)
            hn_v = h_d[:].rearrange("(t p) h -> p t h", p=P)
            a_v = a_d[:].rearrange("(t p) h -> p t h", p=P)
            # Two HWDGE queues in parallel, priority order per queue.  Early
            # chunks are latency-bound (~1.5-2us each regardless of size),
            # so use few, large chunks: os on sync, hn on scalar.  The
            # scalar queue's automatic ACT table load then lands after its
            # dma issues, right before the first tanh needs it.
            # (gpsimd dma_start is SWDGE — avoid.)
            nc.sync.dma_start(os_f[:, 0:4, :], os_v[:, 0:4, :])
            nc.scalar.dma_start(hn_f[:, 0:4, :], hn_v[:, 0:4, :])
            nc.sync.dma_start(hn_f[:, 4:6, :], hn_v[:, 4:6, :])
            nc.sync.dma_start(hn_f[:, 6:8, :], hn_v[:, 6:8, :])
            nc.scalar.dma_start(hn_f[:, 8:13, :], hn_v[:, 8:13, :])
            nc.sync.dma_start(os_f[:, 4:10, :], os_v[:, 4:10, :])
            nc.sync.dma_start(hn_f[:, 13:16, :], hn_v[:, 13:16, :])
            nc.sync.dma_start(os_f[:, 10:16, :], os_v[:, 10:16, :])

            id_bf = constp.tile([P, P], bf16, tag="idb")
            masks.make_identity(nc, id_bf[:])
            ones_bf = constp.tile([P, 1], bf16, tag="ones")
            nc.vector.memset(ones_bf[:], 1.0)

            # persistent bf16 operands
            hn = bigp.tile([P, T, P], bf16, tag="hn")    # [k_in, t, h] natural
            ht = bigp.tile([P, T, P], bf16, tag="ht")    # [h, t, k_in]
            qT = bigp.tile([P, T, P], bf16, tag="qT")    # [h, t, q_in]
            q_nat = bigp.tile([P, T, H], bf16, tag="qnat")
            ot_all = bigp.tile([P, T, H], f32, tag="ot")  # output staging

            # ---- prologue compute ----
            # earliest-needed history tiles converted on DVE (fast); the
            # latest tiles go to the otherwise-idle Pool engine
            nc.vector.tensor_copy(hn[:, 0:4, :], hn_f[:, 0:4, :])
            nc.scalar.activation(q_nat[:, 0:4, :], os_f[:, 0:4, :], AF.Tanh)
            nc.gpsimd.tensor_copy(hn[:, 13:16, :], hn_f[:, 13:16, :])

            # PE warm-up: the tensor engine ramps its clock only after ~3us
            # of continuous work; burn the DMA-wait window with dummy
            # transposes so the first real matmuls run at full speed
            wrm = pstp.tile([P, 8, P], bf16, tag="tsp", name="warm")
            for i in range(24):
                nc.tensor.transpose(wrm[:, i % 8, :], id_bf[:], id_bf[:])
            # a few late-hinted dummies keep the PE clock up through the
            # DMA-wait window until the first real matmuls
            with tc.tile_wait_until(0.003):
                for i in range(6):
                    nc.tensor.transpose(wrm[:, i, :], id_bf[:], id_bf[:])
            with tc.tile_wait_until(0.0042):
                for i in range(6):
                    nc.tensor.transpose(wrm[:, i, :], id_bf[:], id_bf[:])
            wrm_keep = constp.tile([P, 1], bf16, tag="wk")
            nc.vector.tensor_copy(wrm_keep[:], wrm[:, 0, 0:1])

            # one batch of PE transposes through the shared tsp PSUM bank,
            # then one DVE copy out per destination range
            def tp_batch(jobs):
                # jobs: list of (dst_tile, dst_t0, src_tile, src_t0, n)
                tsp = pstp.tile([P, 8, P], bf16, tag="tsp", name="tsp")
                s = 0
                for dst, dt0, src, st0, n in jobs:
                    for i in range(n):
                        nc.tensor.transpose(tsp[:, s + i, :],
                                            src[:, st0 + i, :], id_bf[:])
                    s += n
                s = 0
                for dst, dt0, src, st0, n in jobs:
                    nc.vector.tensor_copy(dst[:, dt0:dt0 + n, :],
                                          tsp[:, s:s + n, :])
                    s += n

            # first MM1 inputs: ht[0:4] (after its DVE cast) and qT[0:4]
            tp_batch([(ht, 0, hn, 0, 4), (qT, 0, q_nat, 0, 4)])

            def groups_of(q):
                return GROUPS_LAST if q == NQ - 1 else GROUPS_EARLY

            # ---- pipeline emitters ----
            sts = {}          # (q, g) -> st tile
            exs = {}          # (q, g) -> ex tile
            accs = [None] * NQ

            def emit_mm1(q, g):
                k0, n = groups_of(q)[g]
                st = psst.tile([P, 3 * QW], f32, tag="st", name=f"st{q}{g}")
                rhs = qT[:, QTPQ * q:QTPQ * (q + 1), :]
                for j in range(n):
                    nc.tensor.matmul(st[:, QW * j:QW * (j + 1)],
                                     ht[:, k0 + j, :], rhs,
                                     start=True, stop=True)
                sts[(q, g)] = st

            def emit_exp(q, g, split=False):
                k0, n = groups_of(q)[g]
                st = sts.pop((q, g))
                ex = expool.tile([P, 3 * QW], bf16, tag="ex", name=f"ex{q}{g}")
                if split:
                    # split the very last exp so the final MM2 (and with it
                    # the whole output tail) starts half an exp earlier
                    nc.scalar.activation(ex[:, 0:QW], st[:, 0:QW], AF.Exp)
                    nc.scalar.activation(ex[:, QW:QW * n], st[:, QW:QW * n],
                                         AF.Exp)
                else:
                    nc.scalar.activation(ex[:, 0:QW * n], st[:, 0:QW * n],
                                         AF.Exp)
                exs[(q, g)] = ex
                dbg_ex.append(ex)

            def emit_mm2(q, g):
                k0, n = groups_of(q)[g]
                if accs[q] is None:
                    accs[q] = pacc.tile([P, QW], f32, tag="acc",
                                        name=f"acc{q}")
                ex = exs[(q, g)]
                for j in range(n):
                    kb = k0 + j
                    nc.tensor.matmul(accs[q][:], hn[:, kb, :],
                                     ex[:, QW * j:QW * (j + 1)],
                                     start=(kb == 0), stop=(kb == T - 1))

            # ---- denominator tree (bf16 adds on DVE) ----
            # state per quarter: pair tiles and the 512-wide running sum
            tstate = {}

            def tadd(name, w, a, b, tag=None, bufs=None):
                # running-sum tiles need bufs=3: S_{i+1} reads S_i, so with
                # 2 bufs the new tile would land on the buffer its own add
                # is reading
                t = treep.tile([P, w], bf16, tag=tag or f"tr{w}", name=name,
                               bufs=bufs)
                nc.vector.tensor_add(t[:], a, b)
                return t

            def sadd(name, a, b):
                return tadd(name, QW, a, b, tag="trS", bufs=3)

            def tree_step(q, g):
                # called after emit_exp(q, g); updates running denominator
                st8 = tstate.setdefault(q, {})
                if q < NQ - 1:
                    # groups 2/2/3/3/3/3
                    if g == 1:
                        st8["t4"] = tadd(f"t4_{q}", 2 * QW,
                                         exs[(q, 0)][:, 0:2 * QW],
                                         exs[(q, 1)][:, 0:2 * QW])
                    elif g == 2:
                        t4 = st8.pop("t4")
                        st8["S"] = sadd(f"s1_{q}", t4[:, 0:QW], t4[:, QW:])
                    elif g == 3:
                        st8["t1"] = tadd(f"t1_{q}", 3 * QW,
                                         exs[(q, 2)][:], exs[(q, 3)][:])
                    elif g == 4:
                        t1 = st8.pop("t1")
                        f1 = tadd(f"f1_{q}", QW, t1[:, 0:QW], t1[:, QW:2 * QW],
                                  tag="trf")
                        s2 = sadd(f"s2_{q}", st8["S"][:], f1[:])
                        st8["S"] = sadd(f"s3_{q}", s2[:], t1[:, 2 * QW:])
                    elif g == 5:
                        st8["t2"] = tadd(f"t2_{q}", 3 * QW,
                                         exs[(q, 4)][:], exs[(q, 5)][:])
                else:
                    # groups 3/3/3/3/2/2.  The denominator is finished by
                    # PE matmul accumulation (the PE has tail slack, DVE
                    # does not): only G0+G1 are folded to a 512-wide S1;
                    # t2 (G2+G3), the G4 fold v, and the G5 fold w feed the
                    # dc accumulation directly.
                    if g == 1:
                        st8["t1"] = tadd(f"t1_{q}", 3 * QW,
                                         exs[(q, 0)][:], exs[(q, 1)][:])
                    elif g == 2:
                        t1 = st8.pop("t1")
                        f1 = tadd(f"f1_{q}", QW, t1[:, 0:QW], t1[:, QW:2 * QW],
                                  tag="trf")
                        st8["S"] = sadd(f"s1_{q}", f1[:], t1[:, 2 * QW:])
                    elif g == 3:
                        st8["t2"] = tadd(f"t2_{q}", 3 * QW,
                                         exs[(q, 2)][:], exs[(q, 3)][:])
                    elif g == 4:
                        t2 = st8.pop("t2")
                        f2 = tadd(f"f2_{q}", QW, t2[:, 0:QW], t2[:, QW:2 * QW],
                                  tag="trf")
                        s2 = sadd(f"s2_{q}", st8["S"][:], f2[:])
                        st8["S"] = sadd(f"s3_{q}", s2[:], t2[:, 2 * QW:])
                    elif g == 5:
                        st8["v"] = tadd(f"v_{q}", QW, exs[(q, 4)][:, 0:QW],
                                        exs[(q, 4)][:, QW:2 * QW], tag="trf")

            def tree_finish(q):
                # mid quarters: fold the last pair tile into the running sum
                st8 = tstate[q]
                t2 = st8.pop("t2")
                f2 = tadd(f"f2_{q}", QW, t2[:, 0:QW], t2[:, QW:2 * QW],
                          tag="trf")
                s4 = sadd(f"s4_{q}", st8["S"][:], f2[:])
                st8["tfin"] = tadd(f"tf_{q}", QW, s4[:], t2[:, 2 * QW:],
                                   tag="tf", bufs=4)
                dbg_tf.append(st8["tfin"])

            def tree_finish_last(q):
                st8 = tstate[q]
                w = tadd(f"w_{q}", QW, exs[(q, 5)][:, 0:QW],
                         exs[(q, 5)][:, QW:2 * QW], tag="trf")
                st8["tfin"] = tadd(f"tf_{q}", QW, st8["S"][:], w[:], tag="tf",
                                   bufs=4)
                dbg_tf.append(st8["tfin"])

            # ---- epilogue helper: one quarter's outputs ----
            def make_epilogue(q):
                st8 = {}

                def dve_copy():
                    aT = workp.tile([P, QW], bf16, tag="at", name=f"aT{q}")
                    if q == NQ - 1:
                        # split between DVE and the now-idle ACT engine so
                        # the tail's transposes start half a copy earlier
                        nc.vector.tensor_copy(aT[:, 0:QW // 2],
                                              accs[q][:, 0:QW // 2])
                        nc.scalar.activation(aT[:, QW // 2:],
                                             accs[q][:, QW // 2:], AF.Copy)
                    else:
                        nc.vector.tensor_copy(aT[:], accs[q][:])
                    st8["aT"] = aT
                    dbg_aT.append(aT)

                def pe_dcols():
                    # d-columns via 1-col matmuls with a ones vector (the
                    # partition-dim sum of tfin).  The dc tile reuses the acc
                    # bank: after the aT copy the accumulator is dead, and
                    # pool WAR ordering (copy -> dc -> recip -> next acc)
                    # keeps PE writes and DVE reads of the bank apart.
                    dc = pacc.tile([P, QW], f32, tag="acc", name=f"dc{q}")
                    tfin = tstate[q]["tfin"]
                    for t in range(QTPQ):
                        nc.tensor.matmul(dc[:, t:t + 1],
                                         tfin[:, P * t:P * (t + 1)],
                                         ones_bf[:], start=True, stop=True)
                    st8["dc"] = dc

                def dve_recip():
                    rc = rcp.tile([P, QTPQ], f32, tag="rc", name=f"rc{q}")
                    nc.vector.reciprocal(rc[:], st8["dc"][:, 0:QTPQ])
                    st8["rc"] = rc
                    rc_tiles.append(rc)

                def pe_transposes():
                    ep = pstp.tile([P, 8, P], bf16, tag="tsp", name=f"ep{q}")
                    st8["ep"] = ep
                    for t in range(QTPQ):
                        nc.tensor.transpose(ep[:, t, :],
                                            st8["aT"][:, P * t:P * (t + 1)],
                                            id_bf[:])

                def dve_muls(ts, on_act=False):
                    for t in ts:
                        if on_act:
                            # ACT is idle after the last exp; Copy-with-scale
                            # halves the tail's serial mul chain
                            nc.scalar.activation(
                                ot_all[:, QTPQ * q + t, :],
                                st8["ep"][:, t, :], AF.Copy,
                                scale=st8["rc"][:, t:t + 1])
                        else:
                            nc.vector.tensor_scalar_mul(
                                ot_all[:, QTPQ * q + t, :],
                                st8["ep"][:, t, :], st8["rc"][:, t:t + 1])

                def dma_out(t0, t1, queue=None):
                    (queue or nc.sync).dma_start(
                        a_v[:, QTPQ * q + t0:QTPQ * q + t1, :],
                        ot_all[:, QTPQ * q + t0:QTPQ * q + t1, :])

                st8["fns"] = (dve_copy, pe_dcols, dve_recip, pe_transposes,
                              dve_muls, dma_out)
                return st8

            # ---- main pipeline ----
            epi = [None] * NQ
            rc_tiles = []
            dbg_ex = []
            dbg_aT = []
            dbg_tf = []

            dc3 = None

            for q in range(NQ):
                last = q == NQ - 1
                for g in range(NG):
                    if q == 0 and g == 0:
                        emit_mm1(0, 0)
                        emit_mm1(0, 1)
                    emit_exp(q, g, split=(last and g == NG - 1))
                    tree_step(q, g)
                    if last and g == NG - 1:
                        # dc_a: accumulate S3 (G0..G3) into a dc tile in the
                        # now-idle st pool while exp(G5) runs; the G4 fold v
                        # and the G5 fold w complete the accumulation later.
                        # Only the FIRST matmul may set start: a start clears
                        # has_written for the WHOLE bank, which would wipe
                        # the other columns' accumulate bits.
                        dc3 = psst.tile([P, 3 * QW], f32, tag="st",
                                        name="dc3")
                        s3 = tstate[q]["S"]
                        for t in range(QTPQ):
                            nc.tensor.matmul(dc3[:, t:t + 1],
                                             s3[:, P * t:P * (t + 1)],
                                             ones_bf[:], start=(t == 0),
                                             stop=False,
                                             skip_group_check=True)

                    if q == 0:
                        # history cast + transpose chains for later groups,
                        # and tanh batches for later quarters; qT(q1) must be
                        # fully emitted before the hoisted MM1(q1, 0) at the
                        # end of slot 4 (PE queue is in-order)
                        if g == 0:
                            with tc.tile_wait_until(0.0055):
                                nc.vector.tensor_copy(hn[:, 4:6, :],
                                                      hn_f[:, 4:6, :])
                            tp_batch([(ht, 4, hn, 4, 2)])
                            with tc.tile_wait_until(0.006):
                                nc.vector.tensor_copy(hn[:, 6:8, :],
                                                      hn_f[:, 6:8, :])
                            tp_batch([(ht, 6, hn, 6, 2)])
                        elif g == 1:
                            with tc.tile_wait_until(0.0055):
                                nc.vector.tensor_copy(hn[:, 8:10, :],
                                                      hn_f[:, 8:10, :])
                            tp_batch([(ht, 8, hn, 8, 2)])
                        elif g == 2:
                            with tc.tile_wait_until(0.0095):
                                nc.vector.tensor_copy(hn[:, 10:13, :],
                                                      hn_f[:, 10:13, :])
                            tp_batch([(ht, 10, hn, 10, 3)])
                        elif g == 3:
                            # one merged tanh for all remaining quarters,
                            # placed after exp(g3) in the ACT stream
                            with tc.tile_wait_until(0.011):
                                nc.scalar.activation(q_nat[:, 4:16, :],
                                                     os_f[:, 4:16, :],
                                                     AF.Tanh)
                            tp_batch([(ht, 13, hn, 13, 3),
                                      (qT, QTPQ, q_nat, QTPQ, 2)])
                        elif g == 4:
                            tp_batch([(qT, QTPQ + 2, q_nat, QTPQ + 2, 2)])
                    elif epi[q - 1] is not None:
                        # epilogue of the previous quarter, spread over slots
                        st8 = epi[q - 1]
                        dve_copy, pe_dcols, dve_recip, pe_trans, dve_muls, \
                            dma_out = st8["fns"]
                        if g == 0:
                            dve_copy()
                        elif g == 1:
                            pe_dcols()
                            dve_recip()
                            # deferred first MM2 of this quarter: its acc
                            # allocation now orders after dc/recip above
                            emit_mm2(q, 0)
                        elif g == 2:
                            pe_trans()
                            dve_muls([0, 1])
                        elif g == 3:
                            dve_muls([2, 3])
                            dma_out(0, 4)
                            epi[q - 1] = None
                            if q < NQ - 1:
                                # before the hoisted MM1(q+1, 0) below
                                tp_batch([(qT, QTPQ * (q + 1), q_nat,
                                           QTPQ * (q + 1), 4)])

                    # leading MM1s, then the lagging MM2 — the scheduler
                    # prefers earlier-priority PE work when both are ready,
                    # and the MM1s feed the exp stream while MM2s have slack
                    if g + 2 < NG:
                        emit_mm1(q, g + 2)
                    elif g + 2 == NG and q < NQ - 1:
                        emit_mm1(q + 1, 0)
                    if g >= 1 and not (g == 1 and q >= 1):
                        # MM2(q, 0) of quarters >= 1 was emitted with the
                        # epilogue extras (acc-bank allocation ordering)
                        emit_mm2(q, g - 1)

                # close the quarter: the hoisted MM1(q+1, 1) goes BEFORE
                # MM2(q, 5) — both wait on exp(q, 5), but the MM1 feeds the
                # next quarter's exp stream while the MM2 has slack
                if not last:
                    emit_mm1(q + 1, 1)
                if last:
                    # dc columns for the G4 fold v: ready before exp(G5)
                    # ends, so they run ahead of MM2(G5) on the PE queue
                    v = tstate[q]["v"]
                    for t in range(QTPQ):
                        nc.tensor.matmul(dc3[:, t:t + 1],
                                         v[:, P * t:P * (t + 1)],
                                         ones_bf[:], start=False, stop=False,
                                         skip_group_check=True)
                emit_mm2(q, NG - 1)
                if not last:
                    tree_finish(q)
                    epi[q] = make_epilogue(q)

            # ---- final quarter tail, emitted tight ----
            q = NQ - 1
            st8 = make_epilogue(q)
            dve_copy, pe_dcols, dve_recip, pe_trans, dve_muls, dma_out = \
                st8["fns"]
            # fold the last group's two chunks, finish the dc accumulation,
            # and take the reciprocal — overlapping the aT copy + transposes
            w = tadd(f"w_{q}", QW, exs[(q, NG - 1)][:, 0:QW],
                     exs[(q, NG - 1)][:, QW:2 * QW], tag="trf")
            dve_copy()
            for t in range(QTPQ):
                nc.tensor.matmul(dc3[:, t:t + 1], w[:, P * t:P * (t + 1)],
                                 ones_bf[:], start=False, stop=(t == QTPQ - 1),
                                 skip_group_check=True)
            rc3 = rcp.tile([P, QTPQ], f32, tag="rc", name="rc3")
            nc.vector.reciprocal(rc3[:], dc3[:, 0:QTPQ])
            st8["rc"] = rc3
            rc_tiles.append(rc3)
            pe_trans()
            dve_muls([0])
            dma_out(0, 1, queue=nc.sync)
            dve_muls([2], on_act=True)
            dma_out(2, 3, queue=nc.scalar)
            dve_muls([1])
            dma_out(1, 2, queue=nc.sync)
            dve_muls([3])
            dma_out(3, 4, queue=nc.scalar)

            if debug_dump:
                for key, src in (("qnat", q_nat), ("qT", qT), ("ht", ht),
                                 ("hn", hn)):
                    nc.sync.dma_start(
                        dbg[key][:].rearrange("(t p) h -> p t h", p=P), src[:])
                nc.sync.dma_start(dbg["attn2"][:].rearrange(
                    "(t p) h -> p t h", p=P), ot_all[:])
                for i, rc in enumerate(rc_tiles):
                    nc.sync.dma_start(dbg["rc"][:, 4 * i:4 * (i + 1)], rc[:])
                for i, at in enumerate(dbg_aT):
                    nc.sync.dma_start(dbg["aT"][:, 512 * i:512 * (i + 1)],
                                      at[:])
                for i, tf in enumerate(dbg_tf):
                    nc.sync.dma_start(dbg["tf"][:, 512 * i:512 * (i + 1)],
                                      tf[:])
                for i, ex in enumerate(dbg_ex):
                    nc.sync.dma_start(dbg["ex"][:, 1536 * i:1536 * (i + 1)],
                                      ex[:])

    nc.compile()
    return nc


def _get_nc():
    if "nc" not in _CACHE:
        _CACHE["nc"] = _build()
    return _CACHE["nc"]


def _run(out_state, history, trace=False):
    from concourse.bass_utils import run_bass_kernel_spmd

    nc = _get_nc()
    out_state = np.ascontiguousarray(out_state, dtype=np.float32)
    history = np.ascontiguousarray(history, dtype=np.float32)
    in_maps = [
        {"out_state": out_state[b], "history": history[b]}
        for b in range(N_CORES)
    ]
    if "warmed" not in _CACHE:
        # The very first execution after NEFF load can start with polluted
        # engine semaphores (the loader's DMA activity bumps them), letting
        # consumers race ahead of producers.  The program's teardown clears
        # all semaphores, so execute once and discard; every execution
        # after that is clean.
        run_bass_kernel_spmd(nc, in_maps, core_ids=list(range(N_CORES)))
        _CACHE["warmed"] = True
    res = run_bass_kernel_spmd(nc, in_maps, core_ids=list(range(N_CORES)),
                               trace=trace)
    attn = np.stack([res.results[b]["attn"] for b in range(N_CORES)], axis=0)
    return attn.astype(np.float32), res


def kernel(out_state, history):
    try:
        attn, _ = _run(out_state, history)
    except Exception:
        # one retry, e.g. if a previous process left a core wedged
        attn, _ = _run(out_state, history)
    return attn


# revision 66
# speedup vs baseline: 1.0331x; 1.0128x over previous
"""Trainium2 Bass kernel for batched tanh-query attention.

Per-batch computation (B=8, one batch per NeuronCore, pure data parallel):
    q = tanh(out_state)            [Q, H]    Q=K=2048, H=128
    S = q @ history.T              [Q, K]
    P = softmax(S, axis=K)
    attn = P @ history             [Q, H]

Flash-style in the transposed orientation S_T[k, q] (no transpose of P
needed for the second matmul). Queries processed in 4 quarters of 512
columns; within a quarter the 16 k-tiles are processed in 6 GROUPS
(2/2/3/3/3/3 tiles) so each exp instruction covers [128, 512*g] —
wider activations amortize the ~352-cycle fixed ACTIVATE overhead
(the Activation engine is the bottleneck: exp at 1 elem/lane/cycle).
    PE : MM1 group g+2 | MM2 group g-1 (accumulating matmuls)
    ACT: exp over st[128, 512*g] f32 PSUM -> ex bf16 SBUF
    DVE: running bf16 adds for the softmax denominator, epilogue
    Pool: f32->bf16 conversion of late history chunks
PSUM budget (8 banks): st x2 (3 banks each) + acc x1 + shared bank
holding both the PE-transpose staging slots and the bf16 denominator
columns (computed by transpose-mode matmuls with a ones vector).
"""

import os
import sys

os.environ.setdefault("NEURON_RT_RESET_CORES", "1")
for _p in ("/opt/trn_rl_repo", "/opt/trn_rl_repo/concourse"):
    if _p not in sys.path:
        sys.path.insert(0, _p)

import numpy as np

N_CORES = 8
SEQ = 2048
H = 128
P = 128
T = SEQ // P          # 16 seq tiles
NQ = 4                # query quarters
QW = SEQ // NQ        # 512
QTPQ = QW // P        # 4 q-tiles per quarter
NG = 6                # k-tile groups per quarter

# group layouts: (start_tile, n_tiles) per group
# early quarters ramp with small groups (fewer ht tiles needed at start);
# the last quarter ends with small groups (short post-exp tail)
GROUPS_EARLY = [(0, 2), (2, 2), (4, 3), (7, 3), (10, 3), (13, 3)]
GROUPS_LAST = [(0, 3), (3, 3), (6, 3), (9, 3), (12, 2), (14, 2)]

_CACHE = {}


def _build():
    from concourse import bacc, bass, masks, mybir, tile

    f32 = mybir.dt.float32
    bf16 = mybir.dt.bfloat16
    AF = mybir.ActivationFunctionType

    debug_dump = os.environ.get("KERNEL_DEBUG_DUMP", "0") == "1"
    EX_BUFS = 24 if debug_dump else 6

    nc = bacc.Bacc("TRN2", target_bir_lowering=False, debug=False,
                   num_devices=N_CORES)
    os_d = nc.dram_tensor("out_state", (SEQ, H), f32, kind="ExternalInput")
    h_d = nc.dram_tensor("history", (SEQ, H), f32, kind="ExternalInput")
    a_d = nc.dram_tensor("attn", (SEQ, H), f32, kind="ExternalOutput")
    if debug_dump:
        dbg = {
            "qnat": nc.dram_tensor("dbg_qnat", (SEQ, H), mybir.dt.bfloat16,
                                   kind="ExternalOutput"),
            "qT": nc.dram_tensor("dbg_qT", (SEQ, H), mybir.dt.bfloat16,
                                 kind="ExternalOutput"),
            "ht": nc.dram_tensor("dbg_ht", (SEQ, H), mybir.dt.bfloat16,
                                 kind="ExternalOutput"),
            "hn": nc.dram_tensor("dbg_hn", (SEQ, H), mybir.dt.bfloat16,
                                 kind="ExternalOutput"),
            "rc": nc.dram_tensor("dbg_rc", (P, 16), f32,
                                 kind="ExternalOutput"),
            "attn2": nc.dram_tensor("dbg_attn2", (SEQ, H), f32,
                                    kind="ExternalOutput"),
            "aT": nc.dram_tensor("dbg_aT", (P, 2048), mybir.dt.bfloat16,
                                 kind="ExternalOutput"),
            "tf": nc.dram_tensor("dbg_tf", (P, 2048), mybir.dt.bfloat16,
                                 kind="ExternalOutput"),
            "ex": nc.dram_tensor("dbg_ex", (P, 24 * 1536), mybir.dt.bfloat16,
                                 kind="ExternalOutput"),
        }

    with tile.TileContext(nc) as tc:
        with (
            tc.tile_pool(name="const", bufs=1) as constp,
            tc.tile_pool(name="big", bufs=1) as bigp,
            tc.tile_pool(name="ex", bufs=EX_BUFS) as expool,
            tc.tile_pool(name="tree", bufs=3) as treep,
            tc.tile_pool(name="work", bufs=4) as workp,
            tc.tile_pool(name="rcp", bufs=4) as rcp,
            tc.tile_pool(name="psst", bufs=2, space=bass.MemorySpace.PSUM) as psst,
            tc.tile_pool(name="pacc", bufs=1, space=bass.MemorySpace.PSUM) as pacc,
            tc.tile_pool(name="pstp", bufs=1, space=bass.MemorySpace.PSUM) as pstp,
        ):
            # ---- input DMAs first so transfers overlap const setup ----
            os_f = bigp.tile([P, T, H], f32, tag="osf")
            hn_f = bigp.tile([P, T, H], f32, tag="hnf")
            os_v = os_d[:].rearrange("(t p) h -> p t h", p=P)
            hn_v = h_d[:].rearrange("(t p) h -> p t h", p=P)
            a_v = a_d[:].rearrange("(t p) h -> p t h", p=P)
            # Two HWDGE queues in parallel, priority order per queue.  Early
            # chunks are latency-bound (~1.5-2us each regardless of size),
            # so use few, large chunks: os on sync, hn on scalar.  The
            # scalar queue's automatic ACT table load then lands after its
            # dma issues, right before the first tanh needs it.
            # (gpsimd dma_start is SWDGE — avoid.)
            nc.sync.dma_start(os_f[:, 0:4, :], os_v[:, 0:4, :])
            nc.scalar.dma_start(hn_f[:, 0:4, :], hn_v[:, 0:4, :])
            nc.sync.dma_start(hn_f[:, 4:6, :], hn_v[:, 4:6, :])
            nc.sync.dma_start(hn_f[:, 6:8, :], hn_v[:, 6:8, :])
            nc.scalar.dma_start(hn_f[:, 8:13, :], hn_v[:, 8:13, :])
            nc.sync.dma_start(os_f[:, 4:10, :], os_v[:, 4:10, :])
            nc.sync.dma_start(hn_f[:, 13:16, :], hn_v[:, 13:16, :])
            nc.sync.dma_start(os_f[:, 10:16, :], os_v[:, 10:16, :])

            id_bf = constp.tile([P, P], bf16, tag="idb")
            masks.make_identity(nc, id_bf[:])
            ones_bf = constp.tile([P, 1], bf16, tag="ones")
            nc.vector.memset(ones_bf[:], 1.0)

            # persistent bf16 operands
            hn = bigp.tile([P, T, P], bf16, tag="hn")    # [k_in, t, h] natural
            ht = bigp.tile([P, T, P], bf16, tag="ht")    # [h, t, k_in]
            qT = bigp.tile([P, T, P], bf16, tag="qT")    # [h, t, q_in]
            q_nat = bigp.tile([P, T, H], bf16, tag="qnat")
            ot_all = bigp.tile([P, T, H], f32, tag="ot")  # output staging

            # ---- prologue compute ----
            # earliest-needed history tiles converted on DVE (fast); the
            # latest tiles go to the otherwise-idle Pool engine
            nc.vector.tensor_copy(hn[:, 0:4, :], hn_f[:, 0:4, :])
            nc.scalar.activation(q_nat[:, 0:4, :], os_f[:, 0:4, :], AF.Tanh)
            nc.gpsimd.tensor_copy(hn[:, 13:16, :], hn_f[:, 13:16, :])

            # PE warm-up: the tensor engine ramps its clock only after ~3us
            # of continuous work; burn the DMA-wait window with dummy
            # transposes so the first real matmuls run at full speed
            wrm = pstp.tile([P, 8, P], bf16, tag="tsp", name="warm")
            for i in range(24):
                nc.tensor.transpose(wrm[:, i % 8, :], id_bf[:], id_bf[:])
            # a few late-hinted dummies keep the PE clock up through the
            # DMA-wait window until the first real matmuls
            with tc.tile_wait_until(0.003):
                for i in range(6):
                    nc.tensor.transpose(wrm[:, i, :], id_bf[:], id_bf[:])
            with tc.tile_wait_until(0.0042):
                for i in range(6):
                    nc.tensor.transpose(wrm[:, i, :], id_bf[:], id_bf[:])
            wrm_keep = constp.tile([P, 1], bf16, tag="wk")
            nc.vector.tensor_copy(wrm_keep[:], wrm[:, 0, 0:1])

            # one batch of PE transposes through the shared tsp PSUM bank,
            # then one DVE copy out per destination range
            def tp_batch(jobs):
                # jobs: list of (dst_tile, dst_t0, src_tile, src_t0, n)
                tsp = pstp.tile([P, 8, P], bf16, tag="tsp", name="tsp")
                s = 0
                for dst, dt0, src, st0, n in jobs:
                    for i in range(n):
                        nc.tensor.transpose(tsp[:, s + i, :],
                                            src[:, st0 + i, :], id_bf[:])
                    s += n
                s = 0
                for dst, dt0, src, st0, n in jobs:
                    nc.vector.tensor_copy(dst[:, dt0:dt0 + n, :],
                                          tsp[:, s:s + n, :])
                    s += n

            # first MM1 inputs: ht[0:4] (after its DVE cast) and qT[0:4]
            tp_batch([(ht, 0, hn, 0, 4), (qT, 0, q_nat, 0, 4)])

            def groups_of(q):
                return GROUPS_LAST if q == NQ - 1 else GROUPS_EARLY

            # ---- pipeline emitters ----
            sts = {}          # (q, g) -> st tile
            exs = {}          # (q, g) -> ex tile
            accs = [None] * NQ

            def emit_mm1(q, g):
                k0, n = groups_of(q)[g]
                st = psst.tile([P, 3 * QW], f32, tag="st", name=f"st{q}{g}")
                rhs = qT[:, QTPQ * q:QTPQ * (q + 1), :]
                for j in range(n):
                    nc.tensor.matmul(st[:, QW * j:QW * (j + 1)],
                                     ht[:, k0 + j, :], rhs,
                                     start=True, stop=True)
                sts[(q, g)] = st

            def emit_exp(q, g, split=False):
                k0, n = groups_of(q)[g]
                st = sts.pop((q, g))
                ex = expool.tile([P, 3 * QW], bf16, tag="ex", name=f"ex{q}{g}")
                if split:
                    # split the very last exp so the final MM2 (and with it
                    # the whole output tail) starts half an exp earlier
                    nc.scalar.activation(ex[:, 0:QW], st[:, 0:QW], AF.Exp)
                    nc.scalar.activation(ex[:, QW:QW * n], st[:, QW:QW * n],
                                         AF.Exp)
                else:
                    nc.scalar.activation(ex[:, 0:QW * n], st[:, 0:QW * n],
                                         AF.Exp)
                exs[(q, g)] = ex
                dbg_ex.append(ex)

            def emit_mm2(q, g):
                k0, n = groups_of(q)[g]
                if accs[q] is None:
                    accs[q] = pacc.tile([P, QW], f32, tag="acc",
                                        name=f"acc{q}")
                ex = exs[(q, g)]
                for j in range(n):
                    kb = k0 + j
                    nc.tensor.matmul(accs[q][:], hn[:, kb, :],
                                     ex[:, QW * j:QW * (j + 1)],
                                     start=(kb == 0), stop=(kb == T - 1))

            # ---- denominator tree (bf16 adds on DVE) ----
            # state per quarter: pair tiles and the 512-wide running sum
            tstate = {}

            def tadd(name, w, a, b, tag=None, bufs=None):
                # running-sum tiles need bufs=3: S_{i+1} reads S_i, so with
                # 2 bufs the new tile would land on the buffer its own add
                # is reading
                t = treep.tile([P, w], bf16, tag=tag or f"tr{w}", name=name,
                               bufs=bufs)
                nc.vector.tensor_add(t[:], a, b)
                return t

            def sadd(name, a, b):
                return tadd(name, QW, a, b, tag="trS", bufs=3)

            def tree_step(q, g):
                # called after emit_exp(q, g); updates running denominator
                st8 = tstate.setdefault(q, {})
                if q < NQ - 1:
                    # groups 2/2/3/3/3/3
                    if g == 1:
                        st8["t4"] = tadd(f"t4_{q}", 2 * QW,
                                         exs[(q, 0)][:, 0:2 * QW],
                                         exs[(q, 1)][:, 0:2 * QW])
                    elif g == 2:
                        t4 = st8.pop("t4")
                        st8["S"] = sadd(f"s1_{q}", t4[:, 0:QW], t4[:, QW:])
                    elif g == 3:
                        st8["t1"] = tadd(f"t1_{q}", 3 * QW,
                                         exs[(q, 2)][:], exs[(q, 3)][:])
                    elif g == 4:
                        t1 = st8.pop("t1")
                        f1 = tadd(f"f1_{q}", QW, t1[:, 0:QW], t1[:, QW:2 * QW],
                                  tag="trf")
                        s2 = sadd(f"s2_{q}", st8["S"][:], f1[:])
                        st8["S"] = sadd(f"s3_{q}", s2[:], t1[:, 2 * QW:])
                    elif g == 5:
                        st8["t2"] = tadd(f"t2_{q}", 3 * QW,
                                         exs[(q, 4)][:], exs[(q, 5)][:])
                else:
                    # groups 3/3/3/3/2/2.  The denominator is finished by
                    # PE matmul accumulation (the PE has tail slack, DVE
                    # does not): only G0+G1 are folded to a 512-wide S1;
                    # t2 (G2+G3), the G4 fold v, and the G5 fold w feed the
                    # dc accumulation directly.
                    if g == 1:
                        st8["t1"] = tadd(f"t1_{q}", 3 * QW,
                                         exs[(q, 0)][:], exs[(q, 1)][:])
                    elif g == 2:
                        t1 = st8.pop("t1")
                        f1 = tadd(f"f1_{q}", QW, t1[:, 0:QW], t1[:, QW:2 * QW],
                                  tag="trf")
                        st8["S"] = sadd(f"s1_{q}", f1[:], t1[:, 2 * QW:])
                    elif g == 3:
                        st8["t2"] = tadd(f"t2_{q}", 3 * QW,
                                         exs[(q, 2)][:], exs[(q, 3)][:])
                    elif g == 4:
                        t2 = st8.pop("t2")
                        f2 = tadd(f"f2_{q}", QW, t2[:, 0:QW], t2[:, QW:2 * QW],
                                  tag="trf")
                        s2 = sadd(f"s2_{q}", st8["S"][:], f2[:])
                        st8["S"] = sadd(f"s3_{q}", s2[:], t2[:, 2 * QW:])
                    elif g == 5:
                        st8["v"] = tadd(f"v_{q}", QW, exs[(q, 4)][:, 0:QW],
                                        exs[(q, 4)][:, QW:2 * QW], tag="trf")

            def tree_finish(q):
                # mid quarters: fold the last pair tile into the running sum
                st8 = tstate[q]
                t2 = st8.pop("t2")
                f2 = tadd(f"f2_{q}", QW, t2[:, 0:QW], t2[:, QW:2 * QW],
                          tag="trf")
                s4 = sadd(f"s4_{q}", st8["S"][:], f2[:])
                st8["tfin"] = tadd(f"tf_{q}", QW, s4[:], t2[:, 2 * QW:],
                                   tag="tf", bufs=4)
                dbg_tf.append(st8["tfin"])

            def tree_finish_last(q):
                st8 = tstate[q]
                w = tadd(f"w_{q}", QW, exs[(q, 5)][:, 0:QW],
                         exs[(q, 5)][:, QW:2 * QW], tag="trf")
                st8["tfin"] = tadd(f"tf_{q}", QW, st8["S"][:], w[:], tag="tf",
                                   bufs=4)
                dbg_tf.append(st8["tfin"])

            # ---- epilogue helper: one quarter's outputs ----
            def make_epilogue(q):
                st8 = {}

                def dve_copy():
                    aT = workp.tile([P, QW], bf16, tag="at", name=f"aT{q}")
                    if q == NQ - 1:
                        # split between DVE and the now-idle ACT engine so
                        # the tail's transposes start half a copy earlier
                        nc.vector.tensor_copy(aT[:, 0:QW // 2],
                                              accs[q][:, 0:QW // 2])
                        nc.scalar.activation(aT[:, QW // 2:],
                                             accs[q][:, QW // 2:], AF.Copy)
                    else:
                        nc.vector.tensor_copy(aT[:], accs[q][:])
                    st8["aT"] = aT
                    dbg_aT.append(aT)

                def pe_dcols():
                    # d-columns via 1-col matmuls with a ones vector (the
                    # partition-dim sum of tfin).  The dc tile reuses the acc
                    # bank: after the aT copy the accumulator is dead, and
                    # pool WAR ordering (copy -> dc -> recip -> next acc)
                    # keeps PE writes and DVE reads of the bank apart.
                    dc = pacc.tile([P, QW], f32, tag="acc", name=f"dc{q}")
                    tfin = tstate[q]["tfin"]
                    for t in range(QTPQ):
                        nc.tensor.matmul(dc[:, t:t + 1],
                                         tfin[:, P * t:P * (t + 1)],
                                         ones_bf[:], start=True, stop=True)
                    st8["dc"] = dc

                def dve_recip():
                    rc = rcp.tile([P, QTPQ], f32, tag="rc", name=f"rc{q}")
                    nc.vector.reciprocal(rc[:], st8["dc"][:, 0:QTPQ])
                    st8["rc"] = rc
                    rc_tiles.append(rc)

                def pe_transposes():
                    ep = pstp.tile([P, 8, P], bf16, tag="tsp", name=f"ep{q}")
                    st8["ep"] = ep
                    for t in range(QTPQ):
                        nc.tensor.transpose(ep[:, t, :],
                                            st8["aT"][:, P * t:P * (t + 1)],
                                            id_bf[:])

                def dve_muls(ts, on_act=False):
                    for t in ts:
                        if on_act:
                            # ACT is idle after the last exp; Copy-with-scale
                            # halves the tail's serial mul chain
                            nc.scalar.activation(
                                ot_all[:, QTPQ * q + t, :],
                                st8["ep"][:, t, :], AF.Copy,
                                scale=st8["rc"][:, t:t + 1])
                        else:
                            nc.vector.tensor_scalar_mul(
                                ot_all[:, QTPQ * q + t, :],
                                st8["ep"][:, t, :], st8["rc"][:, t:t + 1])

                def dma_out(t0, t1, queue=None):
                    (queue or nc.sync).dma_start(
                        a_v[:, QTPQ * q + t0:QTPQ * q + t1, :],
                        ot_all[:, QTPQ * q + t0:QTPQ * q + t1, :])

                st8["fns"] = (dve_copy, pe_dcols, dve_recip, pe_transposes,
                              dve_muls, dma_out)
                return st8

            # ---- main pipeline ----
            epi = [None] * NQ
            rc_tiles = []
            dbg_ex = []
            dbg_aT = []
            dbg_tf = []

            dc3 = None

            for q in range(NQ):
                last = q == NQ - 1
                for g in range(NG):
                    if q == 0 and g == 0:
                        emit_mm1(0, 0)
                        emit_mm1(0, 1)
                    emit_exp(q, g, split=(last and g == NG - 1))
                    tree_step(q, g)
                    if last and g == NG - 1:
                        # dc_a: accumulate S3 (G0..G3) into a dc tile in the
                        # now-idle st pool while exp(G5) runs; the G4 fold v
                        # and the G5 fold w complete the accumulation later.
                        # Only the FIRST matmul may set start: a start clears
                        # has_written for the WHOLE bank, which would wipe
                        # the other columns' accumulate bits.
                        dc3 = psst.tile([P, 3 * QW], f32, tag="st",
                                        name="dc3")
                        s3 = tstate[q]["S"]
                        for t in range(QTPQ):
                            nc.tensor.matmul(dc3[:, t:t + 1],
                                             s3[:, P * t:P * (t + 1)],
                                             ones_bf[:], start=(t == 0),
                                             stop=False,
                                             skip_group_check=True)

                    if q == 0:
                        # history cast + transpose chains for later groups,
                        # and tanh batches for later quarters; qT(q1) must be
                        # fully emitted before the hoisted MM1(q1, 0) at the
                        # end of slot 4 (PE queue is in-order)
                        if g == 0:
                            with tc.tile_wait_until(0.0055):
                                nc.vector.tensor_copy(hn[:, 4:6, :],
                                                      hn_f[:, 4:6, :])
                            tp_batch([(ht, 4, hn, 4, 2)])
                            with tc.tile_wait_until(0.006):
                                nc.vector.tensor_copy(hn[:, 6:8, :],
                                                      hn_f[:, 6:8, :])
                            tp_batch([(ht, 6, hn, 6, 2)])
                        elif g == 1:
                            with tc.tile_wait_until(0.0055):
                                nc.vector.tensor_copy(hn[:, 8:10, :],
                                                      hn_f[:, 8:10, :])
                            tp_batch([(ht, 8, hn, 8, 2)])
                        elif g == 2:
                            with tc.tile_wait_until(0.008):
                                nc.scalar.activation(q_nat[:, 4:10, :],
                                                     os_f[:, 4:10, :],
                                                     AF.Tanh)
                            with tc.tile_wait_until(0.0095):
                                nc.vector.tensor_copy(hn[:, 10:13, :],
                                                      hn_f[:, 10:13, :])
                            tp_batch([(ht, 10, hn, 10, 3)])
                        elif g == 3:
                            tp_batch([(ht, 13, hn, 13, 3),
                                      (qT, QTPQ, q_nat, QTPQ, 2)])
                        elif g == 4:
                            with tc.tile_wait_until(0.014):
                                nc.scalar.activation(q_nat[:, 10:16, :],
                                                     os_f[:, 10:16, :],
                                                     AF.Tanh)
                            tp_batch([(qT, QTPQ + 2, q_nat, QTPQ + 2, 2)])
                    elif epi[q - 1] is not None:
                        # epilogue of the previous quarter, spread over slots
                        st8 = epi[q - 1]
                        dve_copy, pe_dcols, dve_recip, pe_trans, dve_muls, \
                            dma_out = st8["fns"]
                        if g == 0:
                            dve_copy()
                        elif g == 1:
                            pe_dcols()
                            dve_recip()
                            # deferred first MM2 of this quarter: its acc
                            # allocation now orders after dc/recip above
                            emit_mm2(q, 0)
                        elif g == 2:
                            pe_trans()
                            dve_muls([0, 1])
                        elif g == 3:
                            dve_muls([2, 3])
                            dma_out(0, 4)
                            epi[q - 1] = None
                            if q < NQ - 1:
                                # before the hoisted MM1(q+1, 0) below
                                tp_batch([(qT, QTPQ * (q + 1), q_nat,
                                           QTPQ * (q + 1), 4)])

                    # leading MM1s, then the lagging MM2 — the scheduler
                    # prefers earlier-priority PE work when both are ready,
                    # and the MM1s feed the exp stream while MM2s have slack
                    if g + 2 < NG:
                        emit_mm1(q, g + 2)
                    elif g + 2 == NG and q < NQ - 1:
                        emit_mm1(q + 1, 0)
                    if g >= 1 and not (g == 1 and q >= 1):
                        # MM2(q, 0) of quarters >= 1 was emitted with the
                        # epilogue extras (acc-bank allocation ordering)
                        emit_mm2(q, g - 1)

                # close the quarter: the hoisted MM1(q+1, 1) goes BEFORE
                # MM2(q, 5) — both wait on exp(q, 5), but the MM1 feeds the
                # next quarter's exp stream while the MM2 has slack
                if not last:
                    emit_mm1(q + 1, 1)
                if last:
                    # dc columns for the G4 fold v: ready before exp(G5)
                    # ends, so they run ahead of MM2(G5) on the PE queue
                    v = tstate[q]["v"]
                    for t in range(QTPQ):
                        nc.tensor.matmul(dc3[:, t:t + 1],
                                         v[:, P * t:P * (t + 1)],
                                         ones_bf[:], start=False, stop=False,
                                         skip_group_check=True)
                emit_mm2(q, NG - 1)
                if not last:
                    tree_finish(q)
                    epi[q] = make_epilogue(q)

            # ---- final quarter tail, emitted tight ----
            q = NQ - 1
            st8 = make_epilogue(q)
            dve_copy, pe_dcols, dve_recip, pe_trans, dve_muls, dma_out = \
                st8["fns"]
            # fold the last group's two chunks, finish the dc accumulation,
            # and take the reciprocal — overlapping the aT copy + transposes
            w = tadd(f"w_{q}", QW, exs[(q, NG - 1)][:, 0:QW],
                     exs[(q, NG - 1)][:, QW:2 * QW], tag="trf")
            dve_copy()
            for t in range(QTPQ):
                nc.tensor.matmul(dc3[:, t:t + 1], w[:, P * t:P * (t + 1)],
                                 ones_bf[:], start=False, stop=(t == QTPQ - 1),
                                 skip_group_check=True)
            rc3 = rcp.tile([P, QTPQ], f32, tag="rc", name="rc3")
            nc.vector.reciprocal(rc3[:], dc3[:, 0:QTPQ])
            st8["rc"] = rc3
            rc_tiles.append(rc3)
            pe_trans()
            dve_muls([0])
            dma_out(0, 1, queue=nc.sync)
            dve_muls([2], on_act=True)
            dma_out(2, 3, queue=nc.scalar)
            dve_muls([1])
            dma_out(1, 2, queue=nc.sync)
            dve_muls([3])
            dma_out(3, 4, queue=nc.scalar)

            if debug_dump:
                for key, src in (("qnat", q_nat), ("qT", qT), ("ht", ht),
                                 ("hn", hn)):
                    nc.sync.dma_start(
                        dbg[key][:].rearrange("(t p) h -> p t h", p=P), src[:])
                nc.sync.dma_start(dbg["attn2"][:].rearrange(
                    "(t p) h -> p t h", p=P), ot_all[:])
                for i, rc in enumerate(rc_tiles):
                    nc.sync.dma_start(dbg["rc"][:, 4 * i:4 * (i + 1)], rc[:])
                for i, at in enumerate(dbg_aT):
                    nc.sync.dma_start(dbg["aT"][:, 512 * i:512 * (i + 1)],
                                      at[:])
                for i, tf in enumerate(dbg_tf):
                    nc.sync.dma_start(dbg["tf"][:, 512 * i:512 * (i + 1)],
                                      tf[:])
                for i, ex in enumerate(dbg_ex):
                    nc.sync.dma_start(dbg["ex"][:, 1536 * i:1536 * (i + 1)],
                                      ex[:])

    nc.compile()
    return nc


def _get_nc():
    if "nc" not in _CACHE:
        _CACHE["nc"] = _build()
    return _CACHE["nc"]


def _run(out_state, history, trace=False):
    from concourse.bass_utils import run_bass_kernel_spmd

    nc = _get_nc()
    out_state = np.ascontiguousarray(out_state, dtype=np.float32)
    history = np.ascontiguousarray(history, dtype=np.float32)
    in_maps = [
        {"out_state": out_state[b], "history": history[b]}
        for b in range(N_CORES)
    ]
    if "warmed" not in _CACHE:
        # The very first execution after NEFF load can start with polluted
        # engine semaphores (the loader's DMA activity bumps them), letting
        # consumers race ahead of producers.  The program's teardown clears
        # all semaphores, so execute once and discard; every execution
        # after that is clean.
        run_bass_kernel_spmd(nc, in_maps, core_ids=list(range(N_CORES)))
        _CACHE["warmed"] = True
    res = run_bass_kernel_spmd(nc, in_maps, core_ids=list(range(N_CORES)),
                               trace=trace)
    attn = np.stack([res.results[b]["attn"] for b in range(N_CORES)], axis=0)
    return attn.astype(np.float32), res


def kernel(out_state, history):
    try:
        attn, _ = _run(out_state, history)
    except Exception:
        # one retry, e.g. if a previous process left a core wedged
        attn, _ = _run(out_state, history)
    return attn


# revision 67
# speedup vs baseline: 1.0387x; 1.0054x over previous
"""Trainium2 Bass kernel for batched tanh-query attention.

Per-batch computation (B=8, one batch per NeuronCore, pure data parallel):
    q = tanh(out_state)            [Q, H]    Q=K=2048, H=128
    S = q @ history.T              [Q, K]
    P = softmax(S, axis=K)
    attn = P @ history             [Q, H]

Flash-style in the transposed orientation S_T[k, q] (no transpose of P
needed for the second matmul). Queries processed in 4 quarters of 512
columns; within a quarter the 16 k-tiles are processed in 6 GROUPS
(2/2/3/3/3/3 tiles) so each exp instruction covers [128, 512*g] —
wider activations amortize the ~352-cycle fixed ACTIVATE overhead
(the Activation engine is the bottleneck: exp at 1 elem/lane/cycle).
    PE : MM1 group g+2 | MM2 group g-1 (accumulating matmuls)
    ACT: exp over st[128, 512*g] f32 PSUM -> ex bf16 SBUF
    DVE: running bf16 adds for the softmax denominator, epilogue
    Pool: f32->bf16 conversion of late history chunks
PSUM budget (8 banks): st x2 (3 banks each) + acc x1 + shared bank
holding both the PE-transpose staging slots and the bf16 denominator
columns (computed by transpose-mode matmuls with a ones vector).
"""

import os
import sys

os.environ.setdefault("NEURON_RT_RESET_CORES", "1")
for _p in ("/opt/trn_rl_repo", "/opt/trn_rl_repo/concourse"):
    if _p not in sys.path:
        sys.path.insert(0, _p)

import numpy as np

N_CORES = 8
SEQ = 2048
H = 128
P = 128
T = SEQ // P          # 16 seq tiles
NQ = 4                # query quarters
QW = SEQ // NQ        # 512
QTPQ = QW // P        # 4 q-tiles per quarter
NG = 6                # k-tile groups per quarter

# group layouts: (start_tile, n_tiles) per group
# early quarters ramp with small groups (fewer ht tiles needed at start);
# the last quarter ends with small groups (short post-exp tail)
GROUPS_EARLY = [(0, 1), (1, 3), (4, 3), (7, 3), (10, 3), (13, 3)]
GROUPS_LAST = [(0, 3), (3, 3), (6, 3), (9, 3), (12, 2), (14, 2)]

_CACHE = {}


def _build():
    from concourse import bacc, bass, masks, mybir, tile

    f32 = mybir.dt.float32
    bf16 = mybir.dt.bfloat16
    AF = mybir.ActivationFunctionType

    debug_dump = os.environ.get("KERNEL_DEBUG_DUMP", "0") == "1"
    EX_BUFS = 24 if debug_dump else 6

    nc = bacc.Bacc("TRN2", target_bir_lowering=False, debug=False,
                   num_devices=N_CORES)
    os_d = nc.dram_tensor("out_state", (SEQ, H), f32, kind="ExternalInput")
    h_d = nc.dram_tensor("history", (SEQ, H), f32, kind="ExternalInput")
    a_d = nc.dram_tensor("attn", (SEQ, H), f32, kind="ExternalOutput")
    if debug_dump:
        dbg = {
            "qnat": nc.dram_tensor("dbg_qnat", (SEQ, H), mybir.dt.bfloat16,
                                   kind="ExternalOutput"),
            "qT": nc.dram_tensor("dbg_qT", (SEQ, H), mybir.dt.bfloat16,
                                 kind="ExternalOutput"),
            "ht": nc.dram_tensor("dbg_ht", (SEQ, H), mybir.dt.bfloat16,
                                 kind="ExternalOutput"),
            "hn": nc.dram_tensor("dbg_hn", (SEQ, H), mybir.dt.bfloat16,
                                 kind="ExternalOutput"),
            "rc": nc.dram_tensor("dbg_rc", (P, 16), f32,
                                 kind="ExternalOutput"),
            "attn2": nc.dram_tensor("dbg_attn2", (SEQ, H), f32,
                                    kind="ExternalOutput"),
            "aT": nc.dram_tensor("dbg_aT", (P, 2048), mybir.dt.bfloat16,
                                 kind="ExternalOutput"),
            "tf": nc.dram_tensor("dbg_tf", (P, 2048), mybir.dt.bfloat16,
                                 kind="ExternalOutput"),
            "ex": nc.dram_tensor("dbg_ex", (P, 24 * 1536), mybir.dt.bfloat16,
                                 kind="ExternalOutput"),
        }

    with tile.TileContext(nc) as tc:
        with (
            tc.tile_pool(name="const", bufs=1) as constp,
            tc.tile_pool(name="big", bufs=1) as bigp,
            tc.tile_pool(name="ex", bufs=EX_BUFS) as expool,
            tc.tile_pool(name="tree", bufs=3) as treep,
            tc.tile_pool(name="work", bufs=4) as workp,
            tc.tile_pool(name="rcp", bufs=4) as rcp,
            tc.tile_pool(name="psst", bufs=2, space=bass.MemorySpace.PSUM) as psst,
            tc.tile_pool(name="pacc", bufs=1, space=bass.MemorySpace.PSUM) as pacc,
            tc.tile_pool(name="pstp", bufs=1, space=bass.MemorySpace.PSUM) as pstp,
        ):
            # ---- input DMAs first so transfers overlap const setup ----
            os_f = bigp.tile([P, T, H], f32, tag="osf")
            hn_f = bigp.tile([P, T, H], f32, tag="hnf")
            os_v = os_d[:].rearrange("(t p) h -> p t h", p=P)
            hn_v = h_d[:].rearrange("(t p) h -> p t h", p=P)
            a_v = a_d[:].rearrange("(t p) h -> p t h", p=P)
            # Two HWDGE queues in parallel, priority order per queue.  Early
            # chunks are latency-bound (~1.5-2us each regardless of size),
            # so use few, large chunks: os on sync, hn on scalar.  The
            # scalar queue's automatic ACT table load then lands after its
            # dma issues, right before the first tanh needs it.
            # (gpsimd dma_start is SWDGE — avoid.)
            nc.sync.dma_start(os_f[:, 0:4, :], os_v[:, 0:4, :])
            nc.scalar.dma_start(hn_f[:, 0:4, :], hn_v[:, 0:4, :])
            nc.sync.dma_start(hn_f[:, 4:6, :], hn_v[:, 4:6, :])
            nc.sync.dma_start(hn_f[:, 6:8, :], hn_v[:, 6:8, :])
            nc.scalar.dma_start(hn_f[:, 8:13, :], hn_v[:, 8:13, :])
            nc.sync.dma_start(os_f[:, 4:10, :], os_v[:, 4:10, :])
            nc.sync.dma_start(hn_f[:, 13:16, :], hn_v[:, 13:16, :])
            nc.sync.dma_start(os_f[:, 10:16, :], os_v[:, 10:16, :])

            id_bf = constp.tile([P, P], bf16, tag="idb")
            masks.make_identity(nc, id_bf[:])
            ones_bf = constp.tile([P, 1], bf16, tag="ones")
            nc.vector.memset(ones_bf[:], 1.0)

            # persistent bf16 operands
            hn = bigp.tile([P, T, P], bf16, tag="hn")    # [k_in, t, h] natural
            ht = bigp.tile([P, T, P], bf16, tag="ht")    # [h, t, k_in]
            qT = bigp.tile([P, T, P], bf16, tag="qT")    # [h, t, q_in]
            q_nat = bigp.tile([P, T, H], bf16, tag="qnat")
            ot_all = bigp.tile([P, T, H], f32, tag="ot")  # output staging

            # ---- prologue compute ----
            # earliest-needed history tiles converted on DVE (fast); the
            # latest tiles go to the otherwise-idle Pool engine
            nc.vector.tensor_copy(hn[:, 0:4, :], hn_f[:, 0:4, :])
            nc.scalar.activation(q_nat[:, 0:4, :], os_f[:, 0:4, :], AF.Tanh)
            nc.gpsimd.tensor_copy(hn[:, 13:16, :], hn_f[:, 13:16, :])

            # PE warm-up: the tensor engine ramps its clock only after ~3us
            # of continuous work; burn the DMA-wait window with dummy
            # transposes so the first real matmuls run at full speed
            wrm = pstp.tile([P, 8, P], bf16, tag="tsp", name="warm")
            for i in range(24):
                nc.tensor.transpose(wrm[:, i % 8, :], id_bf[:], id_bf[:])
            # a few late-hinted dummies keep the PE clock up through the
            # DMA-wait window until the first real matmuls
            with tc.tile_wait_until(0.003):
                for i in range(6):
                    nc.tensor.transpose(wrm[:, i, :], id_bf[:], id_bf[:])
            with tc.tile_wait_until(0.0042):
                for i in range(6):
                    nc.tensor.transpose(wrm[:, i, :], id_bf[:], id_bf[:])
            wrm_keep = constp.tile([P, 1], bf16, tag="wk")
            nc.vector.tensor_copy(wrm_keep[:], wrm[:, 0, 0:1])

            # one batch of PE transposes through the shared tsp PSUM bank,
            # then one DVE copy out per destination range
            def tp_batch(jobs):
                # jobs: list of (dst_tile, dst_t0, src_tile, src_t0, n)
                tsp = pstp.tile([P, 8, P], bf16, tag="tsp", name="tsp")
                s = 0
                for dst, dt0, src, st0, n in jobs:
                    for i in range(n):
                        nc.tensor.transpose(tsp[:, s + i, :],
                                            src[:, st0 + i, :], id_bf[:])
                    s += n
                s = 0
                for dst, dt0, src, st0, n in jobs:
                    nc.vector.tensor_copy(dst[:, dt0:dt0 + n, :],
                                          tsp[:, s:s + n, :])
                    s += n

            # first MM1 inputs: ht[0:4] (after its DVE cast) and qT[0:4]
            tp_batch([(ht, 0, hn, 0, 4), (qT, 0, q_nat, 0, 4)])

            def groups_of(q):
                return GROUPS_LAST if q == NQ - 1 else GROUPS_EARLY

            # ---- pipeline emitters ----
            sts = {}          # (q, g) -> st tile
            exs = {}          # (q, g) -> ex tile
            accs = [None] * NQ

            def emit_mm1(q, g):
                k0, n = groups_of(q)[g]
                st = psst.tile([P, 3 * QW], f32, tag="st", name=f"st{q}{g}")
                rhs = qT[:, QTPQ * q:QTPQ * (q + 1), :]
                for j in range(n):
                    nc.tensor.matmul(st[:, QW * j:QW * (j + 1)],
                                     ht[:, k0 + j, :], rhs,
                                     start=True, stop=True)
                sts[(q, g)] = st

            def emit_exp(q, g, split=False):
                k0, n = groups_of(q)[g]
                st = sts.pop((q, g))
                ex = expool.tile([P, 3 * QW], bf16, tag="ex", name=f"ex{q}{g}")
                if split:
                    # split the very last exp so the final MM2 (and with it
                    # the whole output tail) starts half an exp earlier
                    nc.scalar.activation(ex[:, 0:QW], st[:, 0:QW], AF.Exp)
                    nc.scalar.activation(ex[:, QW:QW * n], st[:, QW:QW * n],
                                         AF.Exp)
                else:
                    nc.scalar.activation(ex[:, 0:QW * n], st[:, 0:QW * n],
                                         AF.Exp)
                exs[(q, g)] = ex
                dbg_ex.append(ex)

            def emit_mm2(q, g):
                k0, n = groups_of(q)[g]
                if accs[q] is None:
                    accs[q] = pacc.tile([P, QW], f32, tag="acc",
                                        name=f"acc{q}")
                ex = exs[(q, g)]
                for j in range(n):
                    kb = k0 + j
                    nc.tensor.matmul(accs[q][:], hn[:, kb, :],
                                     ex[:, QW * j:QW * (j + 1)],
                                     start=(kb == 0), stop=(kb == T - 1))

            # ---- denominator tree (bf16 adds on DVE) ----
            # state per quarter: pair tiles and the 512-wide running sum
            tstate = {}

            def tadd(name, w, a, b, tag=None, bufs=None):
                # running-sum tiles need bufs=3: S_{i+1} reads S_i, so with
                # 2 bufs the new tile would land on the buffer its own add
                # is reading
                t = treep.tile([P, w], bf16, tag=tag or f"tr{w}", name=name,
                               bufs=bufs)
                nc.vector.tensor_add(t[:], a, b)
                return t

            def sadd(name, a, b):
                return tadd(name, QW, a, b, tag="trS", bufs=3)

            def tree_step(q, g):
                # called after emit_exp(q, g); updates running denominator
                st8 = tstate.setdefault(q, {})
                if q < NQ - 1:
                    # groups 1/3/3/3/3/3: fold G1's chunks, then add the
                    # single-chunk G0 directly into the running sum
                    if g == 1:
                        st8["u"] = tadd(f"u_{q}", QW,
                                        exs[(q, 1)][:, 0:QW],
                                        exs[(q, 1)][:, QW:2 * QW], tag="trf")
                    elif g == 2:
                        u2 = tadd(f"u2_{q}", QW, st8.pop("u")[:],
                                  exs[(q, 1)][:, 2 * QW:], tag="trf")
                        st8["S"] = sadd(f"s1_{q}", u2[:],
                                        exs[(q, 0)][:, 0:QW])
                    elif g == 3:
                        st8["t1"] = tadd(f"t1_{q}", 3 * QW,
                                         exs[(q, 2)][:], exs[(q, 3)][:])
                    elif g == 4:
                        t1 = st8.pop("t1")
                        f1 = tadd(f"f1_{q}", QW, t1[:, 0:QW], t1[:, QW:2 * QW],
                                  tag="trf")
                        s2 = sadd(f"s2_{q}", st8["S"][:], f1[:])
                        st8["S"] = sadd(f"s3_{q}", s2[:], t1[:, 2 * QW:])
                    elif g == 5:
                        st8["t2"] = tadd(f"t2_{q}", 3 * QW,
                                         exs[(q, 4)][:], exs[(q, 5)][:])
                else:
                    # groups 3/3/3/3/2/2.  The denominator is finished by
                    # PE matmul accumulation (the PE has tail slack, DVE
                    # does not): only G0+G1 are folded to a 512-wide S1;
                    # t2 (G2+G3), the G4 fold v, and the G5 fold w feed the
                    # dc accumulation directly.
                    if g == 1:
                        st8["t1"] = tadd(f"t1_{q}", 3 * QW,
                                         exs[(q, 0)][:], exs[(q, 1)][:])
                    elif g == 2:
                        t1 = st8.pop("t1")
                        f1 = tadd(f"f1_{q}", QW, t1[:, 0:QW], t1[:, QW:2 * QW],
                                  tag="trf")
                        st8["S"] = sadd(f"s1_{q}", f1[:], t1[:, 2 * QW:])
                    elif g == 3:
                        st8["t2"] = tadd(f"t2_{q}", 3 * QW,
                                         exs[(q, 2)][:], exs[(q, 3)][:])
                    elif g == 4:
                        t2 = st8.pop("t2")
                        f2 = tadd(f"f2_{q}", QW, t2[:, 0:QW], t2[:, QW:2 * QW],
                                  tag="trf")
                        s2 = sadd(f"s2_{q}", st8["S"][:], f2[:])
                        st8["S"] = sadd(f"s3_{q}", s2[:], t2[:, 2 * QW:])
                    elif g == 5:
                        st8["v"] = tadd(f"v_{q}", QW, exs[(q, 4)][:, 0:QW],
                                        exs[(q, 4)][:, QW:2 * QW], tag="trf")

            def tree_finish(q):
                # mid quarters: fold the last pair tile into the running sum
                st8 = tstate[q]
                t2 = st8.pop("t2")
                f2 = tadd(f"f2_{q}", QW, t2[:, 0:QW], t2[:, QW:2 * QW],
                          tag="trf")
                s4 = sadd(f"s4_{q}", st8["S"][:], f2[:])
                st8["tfin"] = tadd(f"tf_{q}", QW, s4[:], t2[:, 2 * QW:],
                                   tag="tf", bufs=4)
                dbg_tf.append(st8["tfin"])

            def tree_finish_last(q):
                st8 = tstate[q]
                w = tadd(f"w_{q}", QW, exs[(q, 5)][:, 0:QW],
                         exs[(q, 5)][:, QW:2 * QW], tag="trf")
                st8["tfin"] = tadd(f"tf_{q}", QW, st8["S"][:], w[:], tag="tf",
                                   bufs=4)
                dbg_tf.append(st8["tfin"])

            # ---- epilogue helper: one quarter's outputs ----
            def make_epilogue(q):
                st8 = {}

                def dve_copy():
                    aT = workp.tile([P, QW], bf16, tag="at", name=f"aT{q}")
                    if q == NQ - 1:
                        # split between DVE and the now-idle ACT engine so
                        # the tail's transposes start half a copy earlier
                        nc.vector.tensor_copy(aT[:, 0:QW // 2],
                                              accs[q][:, 0:QW // 2])
                        nc.scalar.activation(aT[:, QW // 2:],
                                             accs[q][:, QW // 2:], AF.Copy)
                    else:
                        nc.vector.tensor_copy(aT[:], accs[q][:])
                    st8["aT"] = aT
                    dbg_aT.append(aT)

                def pe_dcols():
                    # d-columns via 1-col matmuls with a ones vector (the
                    # partition-dim sum of tfin).  The dc tile reuses the acc
                    # bank: after the aT copy the accumulator is dead, and
                    # pool WAR ordering (copy -> dc -> recip -> next acc)
                    # keeps PE writes and DVE reads of the bank apart.
                    dc = pacc.tile([P, QW], f32, tag="acc", name=f"dc{q}")
                    tfin = tstate[q]["tfin"]
                    for t in range(QTPQ):
                        nc.tensor.matmul(dc[:, t:t + 1],
                                         tfin[:, P * t:P * (t + 1)],
                                         ones_bf[:], start=True, stop=True)
                    st8["dc"] = dc

                def dve_recip():
                    rc = rcp.tile([P, QTPQ], f32, tag="rc", name=f"rc{q}")
                    nc.vector.reciprocal(rc[:], st8["dc"][:, 0:QTPQ])
                    st8["rc"] = rc
                    rc_tiles.append(rc)

                def pe_transposes():
                    ep = pstp.tile([P, 8, P], bf16, tag="tsp", name=f"ep{q}")
                    st8["ep"] = ep
                    for t in range(QTPQ):
                        nc.tensor.transpose(ep[:, t, :],
                                            st8["aT"][:, P * t:P * (t + 1)],
                                            id_bf[:])

                def dve_muls(ts, on_act=False):
                    for t in ts:
                        if on_act:
                            # ACT is idle after the last exp; Copy-with-scale
                            # halves the tail's serial mul chain
                            nc.scalar.activation(
                                ot_all[:, QTPQ * q + t, :],
                                st8["ep"][:, t, :], AF.Copy,
                                scale=st8["rc"][:, t:t + 1])
                        else:
                            nc.vector.tensor_scalar_mul(
                                ot_all[:, QTPQ * q + t, :],
                                st8["ep"][:, t, :], st8["rc"][:, t:t + 1])

                def dma_out(t0, t1, queue=None):
                    (queue or nc.sync).dma_start(
                        a_v[:, QTPQ * q + t0:QTPQ * q + t1, :],
                        ot_all[:, QTPQ * q + t0:QTPQ * q + t1, :])

                st8["fns"] = (dve_copy, pe_dcols, dve_recip, pe_transposes,
                              dve_muls, dma_out)
                return st8

            # ---- main pipeline ----
            epi = [None] * NQ
            rc_tiles = []
            dbg_ex = []
            dbg_aT = []
            dbg_tf = []

            dc3 = None

            for q in range(NQ):
                last = q == NQ - 1
                for g in range(NG):
                    if q == 0 and g == 0:
                        emit_mm1(0, 0)
                        emit_mm1(0, 1)
                    emit_exp(q, g, split=(last and g == NG - 1))
                    tree_step(q, g)
                    if last and g == NG - 1:
                        # dc_a: accumulate S3 (G0..G3) into a dc tile in the
                        # now-idle st pool while exp(G5) runs; the G4 fold v
                        # and the G5 fold w complete the accumulation later.
                        # Only the FIRST matmul may set start: a start clears
                        # has_written for the WHOLE bank, which would wipe
                        # the other columns' accumulate bits.
                        dc3 = psst.tile([P, 3 * QW], f32, tag="st",
                                        name="dc3")
                        s3 = tstate[q]["S"]
                        for t in range(QTPQ):
                            nc.tensor.matmul(dc3[:, t:t + 1],
                                             s3[:, P * t:P * (t + 1)],
                                             ones_bf[:], start=(t == 0),
                                             stop=False,
                                             skip_group_check=True)

                    if q == 0:
                        # history cast + transpose chains for later groups,
                        # and tanh batches for later quarters; qT(q1) must be
                        # fully emitted before the hoisted MM1(q1, 0) at the
                        # end of slot 4 (PE queue is in-order)
                        if g == 0:
                            with tc.tile_wait_until(0.0055):
                                nc.vector.tensor_copy(hn[:, 4:6, :],
                                                      hn_f[:, 4:6, :])
                            tp_batch([(ht, 4, hn, 4, 2)])
                            with tc.tile_wait_until(0.006):
                                nc.vector.tensor_copy(hn[:, 6:8, :],
                                                      hn_f[:, 6:8, :])
                            tp_batch([(ht, 6, hn, 6, 2)])
                        elif g == 1:
                            with tc.tile_wait_until(0.0055):
                                nc.vector.tensor_copy(hn[:, 8:10, :],
                                                      hn_f[:, 8:10, :])
                            tp_batch([(ht, 8, hn, 8, 2)])
                        elif g == 2:
                            with tc.tile_wait_until(0.008):
                                nc.scalar.activation(q_nat[:, 4:10, :],
                                                     os_f[:, 4:10, :],
                                                     AF.Tanh)
                            with tc.tile_wait_until(0.0095):
                                nc.vector.tensor_copy(hn[:, 10:13, :],
                                                      hn_f[:, 10:13, :])
                            tp_batch([(ht, 10, hn, 10, 3)])
                        elif g == 3:
                            tp_batch([(ht, 13, hn, 13, 3),
                                      (qT, QTPQ, q_nat, QTPQ, 2)])
                        elif g == 4:
                            with tc.tile_wait_until(0.014):
                                nc.scalar.activation(q_nat[:, 10:16, :],
                                                     os_f[:, 10:16, :],
                                                     AF.Tanh)
                            tp_batch([(qT, QTPQ + 2, q_nat, QTPQ + 2, 2)])
                    elif epi[q - 1] is not None:
                        # epilogue of the previous quarter, spread over slots
                        st8 = epi[q - 1]
                        dve_copy, pe_dcols, dve_recip, pe_trans, dve_muls, \
                            dma_out = st8["fns"]
                        if g == 0:
                            dve_copy()
                        elif g == 1:
                            pe_dcols()
                            dve_recip()
                            # deferred first MM2 of this quarter: its acc
                            # allocation now orders after dc/recip above
                            emit_mm2(q, 0)
                        elif g == 2:
                            pe_trans()
                            dve_muls([0, 1])
                        elif g == 3:
                            dve_muls([2, 3])
                            dma_out(0, 4)
                            epi[q - 1] = None
                            if q < NQ - 1:
                                # before the hoisted MM1(q+1, 0) below
                                tp_batch([(qT, QTPQ * (q + 1), q_nat,
                                           QTPQ * (q + 1), 4)])

                    # leading MM1s, then the lagging MM2 — the scheduler
                    # prefers earlier-priority PE work when both are ready,
                    # and the MM1s feed the exp stream while MM2s have slack
                    if g + 2 < NG:
                        emit_mm1(q, g + 2)
                    elif g + 2 == NG and q < NQ - 1:
                        emit_mm1(q + 1, 0)
                    if g >= 1 and not (g == 1 and q >= 1):
                        # MM2(q, 0) of quarters >= 1 was emitted with the
                        # epilogue extras (acc-bank allocation ordering)
                        emit_mm2(q, g - 1)

                # close the quarter: the hoisted MM1(q+1, 1) goes BEFORE
                # MM2(q, 5) — both wait on exp(q, 5), but the MM1 feeds the
                # next quarter's exp stream while the MM2 has slack
                if not last:
                    emit_mm1(q + 1, 1)
                if last:
                    # dc columns for the G4 fold v: ready before exp(G5)
                    # ends, so they run ahead of MM2(G5) on the PE queue
                    v = tstate[q]["v"]
                    for t in range(QTPQ):
                        nc.tensor.matmul(dc3[:, t:t + 1],
                                         v[:, P * t:P * (t + 1)],
                                         ones_bf[:], start=False, stop=False,
                                         skip_group_check=True)
                emit_mm2(q, NG - 1)
                if not last:
                    tree_finish(q)
                    epi[q] = make_epilogue(q)

            # ---- final quarter tail, emitted tight ----
            q = NQ - 1
            st8 = make_epilogue(q)
            dve_copy, pe_dcols, dve_recip, pe_trans, dve_muls, dma_out = \
                st8["fns"]
            # fold the last group's two chunks, finish the dc accumulation,
            # and take the reciprocal — overlapping the aT copy + transposes
            w = tadd(f"w_{q}", QW, exs[(q, NG - 1)][:, 0:QW],
                     exs[(q, NG - 1)][:, QW:2 * QW], tag="trf")
            dve_copy()
            for t in range(QTPQ):
                nc.tensor.matmul(dc3[:, t:t + 1], w[:, P * t:P * (t + 1)],
                                 ones_bf[:], start=False, stop=(t == QTPQ - 1),
                                 skip_group_check=True)
            rc3 = rcp.tile([P, QTPQ], f32, tag="rc", name="rc3")
            nc.vector.reciprocal(rc3[:], dc3[:, 0:QTPQ])
            st8["rc"] = rc3
            rc_tiles.append(rc3)
            pe_trans()
            dve_muls([0])
            dma_out(0, 1, queue=nc.sync)
            dve_muls([2], on_act=True)
            dma_out(2, 3, queue=nc.scalar)
            dve_muls([1])
            dma_out(1, 2, queue=nc.sync)
            dve_muls([3])
            dma_out(3, 4, queue=nc.scalar)

            if debug_dump:
                for key, src in (("qnat", q_nat), ("qT", qT), ("ht", ht),
                                 ("hn", hn)):
                    nc.sync.dma_start(
                        dbg[key][:].rearrange("(t p) h -> p t h", p=P), src[:])
                nc.sync.dma_start(dbg["attn2"][:].rearrange(
                    "(t p) h -> p t h", p=P), ot_all[:])
                for i, rc in enumerate(rc_tiles):
                    nc.sync.dma_start(dbg["rc"][:, 4 * i:4 * (i + 1)], rc[:])
                for i, at in enumerate(dbg_aT):
                    nc.sync.dma_start(dbg["aT"][:, 512 * i:512 * (i + 1)],
                                      at[:])
                for i, tf in enumerate(dbg_tf):
                    nc.sync.dma_start(dbg["tf"][:, 512 * i:512 * (i + 1)],
                                      tf[:])
                for i, ex in enumerate(dbg_ex):
                    nc.sync.dma_start(dbg["ex"][:, 1536 * i:1536 * (i + 1)],
                                      ex[:])

    nc.compile()
    return nc


def _get_nc():
    if "nc" not in _CACHE:
        _CACHE["nc"] = _build()
    return _CACHE["nc"]


def _run(out_state, history, trace=False):
    from concourse.bass_utils import run_bass_kernel_spmd

    nc = _get_nc()
    out_state = np.ascontiguousarray(out_state, dtype=np.float32)
    history = np.ascontiguousarray(history, dtype=np.float32)
    in_maps = [
        {"out_state": out_state[b], "history": history[b]}
        for b in range(N_CORES)
    ]
    if "warmed" not in _CACHE:
        # The very first execution after NEFF load can start with polluted
        # engine semaphores (the loader's DMA activity bumps them), letting
        # consumers race ahead of producers.  The program's teardown clears
        # all semaphores, so execute once and discard; every execution
        # after that is clean.
        run_bass_kernel_spmd(nc, in_maps, core_ids=list(range(N_CORES)))
        _CACHE["warmed"] = True
    res = run_bass_kernel_spmd(nc, in_maps, core_ids=list(range(N_CORES)),
                               trace=trace)
    attn = np.stack([res.results[b]["attn"] for b in range(N_CORES)], axis=0)
    return attn.astype(np.float32), res


def kernel(out_state, history):
    try:
        attn, _ = _run(out_state, history)
    except Exception:
        # one retry, e.g. if a previous process left a core wedged
        attn, _ = _run(out_state, history)
    return attn
